# revision 39
# baseline (speedup 1.0000x reference)
"""Trainium2 Bass kernel for nn_LuminaLM (4-layer GPT-2-like transformer + LM head).

Strategy: 8-way Megatron tensor parallel with sequence-parallel residual,
TOKEN-MAJOR residual layout.
 - Each core owns 2 of 16 heads, 1/8 of the vocab; MLP is token-local.
 - Residual h is token-sharded AND token-major: core r owns tokens
   [128r,128r+128) of each batch, stored as [128(tokens), 1024(d)] fp32.
   LayerNorm is a per-partition free-axis reduction (bn_stats/bn_aggr on DVE)
   -> no PE stats matmuls, no cross-partition broadcast chains.
 - LN gains/biases folded into consuming weights/biases on the HOST.
 - AllGather payload is rank-major [128p, dt*t] so both the bounce write and
   the gathered readback use large contiguous DMA descriptors; the per-rank
   local transpose (token-major -> feature-major) is 8 cheap PE transposes.
 - qkv: weights stationary, dti-outer so each LDWEIGHTS covers 2 N=512 MMs.
 - Attention: S^T = k^T q per key tile, exp on ScalarE, causal masking via
   affine_select on diagonal tiles, AV with ones-augmented v; normalization
   on yT eviction with a PE-broadcast reciprocal (per half, pipelined).
 - proj / fc2 use the STATIONARY-ACTIVATION trick (lhsT = yT / mT tile,
   weights stream as rhs) so outputs land token-major [t, d]: residual adds,
   RS payloads and LN stay in token-major layout with contiguous DMAs.
 - ReduceScatter per half (2MB bf16), input written with 2KB-contiguous rows.
 - LM head: vocab-sharded; dti-outer accumulation in 4 PSUM banks so each
   LDWEIGHTS covers 4 N=512 streams; logits written bf16 (upcast on host).
Matmuls bf16 with fp32 PSUM accumulation; collectives ride bf16.
"""

import os
import numpy as np

B, T, D, V, L = 2, 1024, 1024, 32000, 4
H, HD = 16, 64
NCORES = 8
P = 128
TPC = T // NCORES          # 128 tokens per core per batch
HPC = H // NCORES          # 2 heads per core
QKVC = 3 * P               # 384 qkv cols per core
VPC = V // NCORES          # 4000 vocab per core
MC = 128                   # lm-head M chunk (full PE width)
NMC = 32                   # 32 chunks of 128 = 4096 (zero-padded past 4000)
VPAD = MC * NMC            # 4096
DT = D // P                # 8 d-tiles
NFC = 4 * D // P           # 32 fc1-output chunks
NT = T // 512              # 2 query chunks of 512
EPS = 1e-5
ATT_SCALE = 1.0 / np.sqrt(HD)
HD1 = HD + 1

_CACHE = {}
last_exec_time_ns = None
last_result = None


def _build_nc(no_coll=False):
    import concourse.bass as bass
    import concourse.mybir as mybir
    import concourse.tile as tile
    from concourse import bacc
    from concourse.masks import make_identity
    from concourse.bass import IndirectOffsetOnAxis

    dt = mybir.dt
    AF = mybir.ActivationFunctionType
    OP = mybir.AluOpType

    nc = bacc.Bacc("TRN2", target_bir_lowering=False, debug=False,
                   num_devices=NCORES)

    # ---- external parameters (per-core shards, staged by host) ----
    ids_p = nc.declare_dram_parameter("ids_st", [TPC, B], dt.int32, isOutput=False)
    wte_p = nc.declare_dram_parameter("wte", [V, D], dt.float32, isOutput=False)
    wpe_p = nc.declare_dram_parameter("wpe_sh", [TPC, D], dt.float32, isOutput=False)
    wqkv_p = nc.declare_dram_parameter("wqkv_sh", [L, P, DT, QKVC], dt.bfloat16, isOutput=False)
    bqkv_p = nc.declare_dram_parameter("bqkv_sh", [L, P, 3], dt.float32, isOutput=False)
    wproj_p = nc.declare_dram_parameter("wproj_sh", [L, P, D], dt.bfloat16, isOutput=False)
    bproj8_p = nc.declare_dram_parameter("bproj8_bc", [L, P, D], dt.bfloat16, isOutput=False)
    wfc1_p = nc.declare_dram_parameter("wfc1_st", [L, NFC, P, DT, P], dt.bfloat16, isOutput=False)
    bfc1_p = nc.declare_dram_parameter("bfc1_st", [L, P, NFC], dt.float32, isOutput=False)
    wfc2_p = nc.declare_dram_parameter("wfc2_st", [L, NFC, P, D], dt.bfloat16, isOutput=False)
    bfc2_p = nc.declare_dram_parameter("bfc2_bc", [L, P, D], dt.bfloat16, isOutput=False)
    wlm_p = nc.declare_dram_parameter("wlm_st", [NMC, P, DT, MC], dt.bfloat16, isOutput=False)
    blm_p = nc.declare_dram_parameter("blm_st", [MC, NMC], dt.float32, isOutput=False)
    logits_p = nc.declare_dram_parameter("logits", [VPAD, B * T], dt.bfloat16, isOutput=True)

    RG = [list(range(NCORES))]

    with tile.TileContext(nc) as tc:
        with (
            tc.tile_pool(name="const", bufs=1) as cp,
            tc.tile_pool(name="wp", bufs=2) as wp,
            tc.tile_pool(name="ws", bufs=2) as ws,
            tc.tile_pool(name="ap", bufs=2) as app,
            tc.tile_pool(name="lnp", bufs=4) as lnp,
            tc.tile_pool(name="psA", bufs=4, space="PSUM") as psA,
            tc.tile_pool(name="psS", bufs=2, space="PSUM") as psS,
            tc.tile_pool(name="psP", bufs=2, space="PSUM") as psP,
            tc.tile_pool(name="dram", bufs=2, space="DRAM") as dramp,
        ):
            # ---------------- warmup collective ----------------
            warm_in = dramp.tile([P, 2], dt.bfloat16, name="warm_in", tag="wrm")
            warm_out = dramp.tile([NCORES * P, 2], dt.bfloat16, name="warm_out",
                                  tag="wrmo", addr_space="Shared")
            warm_sb = cp.tile([P, 2], dt.bfloat16, name="warm_sb")
            nc.vector.memset(warm_sb[:], 0.0)
            nc.sync.dma_start(warm_in[:], warm_sb[:])
            if no_coll:
                nc.sync.dma_start(warm_out[0:P, :], warm_in[:])
            else:
                nc.gpsimd.collective_compute(
                    "AllGather", OP.bypass, replica_groups=RG,
                    ins=[warm_in[:].opt()], outs=[warm_out[:].opt()],
                )

            # ---------------- constants ----------------
            ident_bf = cp.tile([P, P], dt.bfloat16)
            make_identity(nc, ident_bf[:])
            ones_row_bf = cp.tile([1, P], dt.bfloat16)
            nc.vector.memset(ones_row_bf[:], 1.0)
            eps_t = cp.tile([P, 1], dt.float32)
            nc.vector.memset(eps_t[:], EPS)

            # per-layer small bias tiles
            bqkvt, bfc1t = [], []
            for li in range(L):
                t_ = cp.tile([P, 3], dt.float32, name=f"bqkv{li}")
                nc.sync.dma_start(t_[:], bqkv_p[li])
                bqkvt.append(t_)
                t_ = cp.tile([P, NFC], dt.float32, name=f"bfc1{li}")
                nc.sync.dma_start(t_[:], bfc1_p[li])
                bfc1t.append(t_)
            # broadcast residual-bias tiles (bf16, streamed per layer)
            def load_biasbc(li):
                t1_ = ws.tile([P, D], dt.bfloat16, name=f"bproj8_{li}",
                              tag="biasbc", bufs=4)
                nc.sync.dma_start(t1_[:], bproj8_p[li])
                t2_ = ws.tile([P, D], dt.bfloat16, name=f"bfc2_{li}",
                              tag="biasbc", bufs=4)
                nc.sync.dma_start(t2_[:], bfc2_p[li])
                return t1_, t2_
            blm_all = cp.tile([MC, NMC], dt.float32, name="blm_all")
            nc.sync.dma_start(blm_all[:], blm_p[:])

            wpe_tok = cp.tile([TPC, D], dt.float32)
            nc.sync.dma_start(wpe_tok[:], wpe_p[:])
            idx_sb = cp.tile([TPC, B], dt.int32)
            nc.sync.dma_start(idx_sb[:], ids_p[:])

            # residual, token-major fp32
            hres = [cp.tile([TPC, D], dt.float32, name=f"hres{h}") for h in range(B)]

            # ---------------- LN helpers (token-major) ----------------
            def ln_scale(h_ap, name):
                """mean/var over the free (d) axis -> (rstd, mrstd) [P,1]."""
                bs = lnp.tile([P, 2, 6], dt.float32, name=f"bs_{name}", tag="bs")
                hv = h_ap.rearrange("p (c f) -> p c f", f=512)
                for c in range(2):
                    nc.vector.bn_stats(bs[:, c, :], hv[:, c, :])
                ma = lnp.tile([P, 2], dt.float32, name=f"ma_{name}", tag="ma")
                nc.vector.bn_aggr(ma[:], bs[:])
                std = lnp.tile([P, 1], dt.float32, name=f"std_{name}", tag="std")
                nc.scalar.activation(std[:], ma[:, 1:2], AF.Sqrt, bias=eps_t[:])
                rstd = lnp.tile([P, 1], dt.float32, name=f"rstd_{name}", tag="rstd")
                nc.vector.reciprocal_approx_fast(rstd[:], std[:])
                mrstd = lnp.tile([P, 1], dt.float32, name=f"mrstd_{name}", tag="mrstd")
                nc.vector.scalar_tensor_tensor(
                    out=mrstd[:], in0=ma[:, 0:1], scalar=-1.0, in1=rstd[:],
                    op0=OP.mult, op1=OP.mult)
                return rstd, mrstd

            def normalize_transpose(h_ap, hnT, name):
                """LN h_ap [t,d] -> bf16, transpose to hnT [p, dt, t]."""
                rstd, mrstd = ln_scale(h_ap, name)
                hn = app.tile([TPC, D], dt.bfloat16, name=f"hn_{name}", tag="hn")
                nc.vector.tensor_scalar(hn[:], h_ap, rstd[:], mrstd[:],
                                        OP.mult, OP.add)
                for dc in range(DT):
                    pst = psA.tile([P, P], dt.bfloat16, space="PSUM",
                                   name=f"pstn_{name}", tag="psA")
                    nc.tensor.transpose(pst[:], hn[:, dc * P:(dc + 1) * P],
                                        ident_bf[:])
                    nc.vector.tensor_copy(hnT[:, dc, :], pst[:])

            # ---------------- AG (both halves combined) ----------------
            def emit_ag(name):
                """LN1+transpose+bounce+ONE AG; returns ag_out [8P, B*DT*TPC]."""
                ag_in = dramp.tile([P, B, DT * TPC], dt.bfloat16,
                                   name=f"agin_{name}", tag="agin")
                for h in range(B):
                    hnT = app.tile([P, DT, TPC], dt.bfloat16,
                                   name=f"hnT_{name}{h}", tag="hnT")
                    normalize_transpose(hres[h][:], hnT, f"{name}h{h}")
                    nc.sync.dma_start(ag_in[:, h, :], hnT[:])
                ag_out = dramp.tile([NCORES * P, B * DT * TPC], dt.bfloat16,
                                    name=f"agout_{name}", tag="agout",
                                    addr_space="Shared")
                if no_coll:
                    nc.sync.dma_start(ag_out[0:P, :], ag_in[:])
                else:
                    nc.gpsimd.collective_compute(
                        "AllGather", OP.bypass, replica_groups=RG,
                        ins=[ag_in[:].opt()], outs=[ag_out[:].opt()],
                    )
                return ag_out

            def ag_read(ag_out, name):
                """Read gathered acts as aT[h] [p, r, dt, t] (2KB descriptors).
                4 DMAs: rank-groups 0-3 / 4-7 per half so qkv's tk=0 work can
                start while the second half of the payload is still landing."""
                ag_v = ag_out[:].rearrange("(r p) (b f) -> p r b f", p=P, b=B)
                aths = [app.tile([P, NCORES, DT, TPC], dt.bfloat16,
                                 name=f"aT_{name}{h}", tag="aT")
                        for h in range(B)]
                for h in range(B):
                    for rg in range(2):
                        nc.sync.dma_start(
                            aths[h][:, 4 * rg:4 * rg + 4, :, :],
                            ag_v[:, 4 * rg:4 * rg + 4, h, :])
                return aths

            # ---------------- qkv (per half; LDWEIGHTS shared over 2 tk) ----
            def qkv_block(aTh, wqkv, bqkv, half):
                qkvT = app.tile([P, 3, T], dt.bfloat16, name=f"qkvT{half}",
                                tag="qkvT")
                for c in (2, 1, 0):          # v first so v_aug can start early
                    pss = [psP.tile([P, 512], dt.float32, space="PSUM",
                                    name=f"ps_qkv{half}_{c}_{tk}", tag="psP")
                           for tk in range(NT)]
                    for dti in range(DT):
                        for tk in range(NT):
                            nc.tensor.matmul(
                                pss[tk][:],
                                lhsT=wqkv[:, dti, c * P:(c + 1) * P],
                                rhs=aTh[:, 4 * tk:4 * tk + 4, dti, :],
                                start=(dti == 0), stop=(dti == DT - 1),
                                skip_group_check=True)
                    for tk in range(NT):
                        nc.vector.tensor_scalar_add(
                            qkvT[:, c, tk * 512:(tk + 1) * 512], pss[tk][:],
                            bqkv[:, c:c + 1])
                return qkvT

            # ---------------- attention (per half) ----------------
            def attention_half(qkvT, half):
                v_aug = app.tile([P, DT, HPC * HD1], dt.bfloat16,
                                 name=f"vaug{half}", tag="vaug")
                for h2 in range(HPC):
                    nc.vector.memset(
                        v_aug[:, :, h2 * HD1 + HD:h2 * HD1 + HD1], 1.0)
                for tt in range(DT):
                    pst = psA.tile([P, P], dt.bfloat16, space="PSUM",
                                   name=f"pst_v{half}", tag="psA")
                    nc.tensor.transpose(
                        pst[:], qkvT[:, 2, tt * P:(tt + 1) * P], ident_bf[:])
                    for h2 in range(HPC):
                        nc.vector.tensor_copy(
                            v_aug[:, tt, h2 * HD1:h2 * HD1 + HD],
                            pst[:, h2 * HD:(h2 + 1) * HD])

                yT = app.tile([P, T], dt.bfloat16, name=f"yT{half}", tag="yT")
                tails = []

                def emit_tail(ps_y, hs, qc):
                    den = app.tile([1, 512], dt.bfloat16, name="den", tag="den")
                    nc.vector.tensor_copy(den[:], ps_y[HD:HD1, :])
                    ps_bc = psA.tile([HD, 512], dt.float32, space="PSUM",
                                     name="ps_bc", tag="psA")
                    nc.tensor.matmul(ps_bc[:], lhsT=ones_row_bf[:, :HD],
                                     rhs=den[:], start=True, stop=True)
                    recb = app.tile([HD, 512], dt.float32, name="recb", tag="recb")
                    nc.vector.reciprocal_approx_fast(recb[:], ps_bc[:])
                    nc.vector.tensor_tensor(
                        out=yT[hs:hs + HD, qc * 512:(qc + 1) * 512],
                        in0=ps_y[:HD, :], in1=recb[:], op=OP.mult)

                def emit_av(PTt, h2, qc, nkt):
                    ps_y = psA.tile([HD1, 512], dt.float32, space="PSUM",
                                    name="ps_y", tag="psA")
                    for kt in range(nkt):
                        qlo = max(0, kt * P - qc * 512)
                        nc.tensor.matmul(
                            ps_y[:, qlo:512],
                            lhsT=v_aug[:, kt, h2 * HD1:(h2 + 1) * HD1],
                            rhs=PTt[:, kt, qlo:512],
                            start=(kt == 0), stop=(kt == nkt - 1))
                    tails.append((ps_y, h2 * HD, qc))
                    if len(tails) >= 2:
                        emit_tail(*tails.pop(0))

                prev = None
                for qc in range(NT):
                    for h2 in range(HPC):
                        hs = h2 * HD
                        nkt = qc * 4 + 4
                        PTt = app.tile([P, 8, 512], dt.bfloat16,
                                       name=f"PT{half}_{qc}_{h2}", tag="PT",
                                       bufs=2)
                        for kt in range(nkt):
                            qlo = max(0, kt * P - qc * 512)
                            ps_st = psS.tile([P, 512], dt.float32,
                                             space="PSUM", name="ps_st",
                                             tag="psS")
                            nc.tensor.matmul(
                                ps_st[:, qlo:512],
                                lhsT=qkvT[hs:hs + HD, 1, kt * P:(kt + 1) * P],
                                rhs=qkvT[hs:hs + HD, 0,
                                         qc * 512 + qlo:(qc + 1) * 512],
                                start=True, stop=True)
                            nc.scalar.activation(
                                PTt[:, kt, qlo:512], ps_st[:, qlo:512],
                                AF.Exp, scale=ATT_SCALE)
                            if kt >= qc * 4:
                                nc.gpsimd.affine_select(
                                    out=PTt[:, kt, qlo:qlo + P],
                                    in_=PTt[:, kt, qlo:qlo + P],
                                    compare_op=OP.is_ge, fill=0.0, base=0,
                                    pattern=[[1, P]], channel_multiplier=-1)
                        if prev is not None:
                            emit_av(*prev)
                        prev = (PTt, h2, qc, nkt)
                emit_av(*prev)
                while tails:
                    emit_tail(*tails.pop(0))
                return yT

            # ---------------- proj + RS (per half) ----------------
            def proj_rs(yT, wproj, bproj8, half, name):
                """proj partial token-major via stationary-yT; write rs_in."""
                rs_in = dramp.tile([T, D], dt.bfloat16, name=f"rsin_{name}",
                                   tag="rsin")
                for tt in range(DT):
                    prc = app.tile([P, D], dt.bfloat16, name="prc", tag="prc",
                                   bufs=4)
                    for dc2 in range(2):
                        ps = psP.tile([P, 512], dt.float32, space="PSUM",
                                      name="ps_pr", tag="psP")
                        nc.tensor.matmul(
                            ps[:], lhsT=yT[:, tt * P:(tt + 1) * P],
                            rhs=wproj[:, dc2 * 512:(dc2 + 1) * 512],
                            start=True, stop=True)
                        nc.vector.tensor_tensor(
                            out=prc[:, dc2 * 512:(dc2 + 1) * 512], in0=ps[:],
                            in1=bproj8[:, dc2 * 512:(dc2 + 1) * 512],
                            op=OP.add)
                    nc.gpsimd.dma_start(rs_in[tt * P:(tt + 1) * P, :], prc[:])
                rs_out = dramp.tile([TPC, D], dt.bfloat16, name=f"rsout_{name}",
                                    tag="rsout")
                if no_coll:
                    nc.sync.dma_start(rs_out[:], rs_in[0:TPC, :])
                else:
                    nc.gpsimd.collective_compute(
                        "ReduceScatter", OP.add, replica_groups=RG,
                        ins=[rs_in[:].opt()], outs=[rs_out[:].opt()],
                    )
                return rs_out

            # ---------------- embedding ----------------
            for h in range(B):
                nc.gpsimd.indirect_dma_start(
                    out=hres[h][:], out_offset=None, in_=wte_p[:],
                    in_offset=IndirectOffsetOnAxis(ap=idx_sb[:, h:h + 1], axis=0),
                )
                nc.vector.tensor_add(hres[h][:], hres[h][:], wpe_tok[:])

            # weights for layer 0
            wqkv_t = wp.tile([P, DT, QKVC], dt.bfloat16, name="wqkv0", tag="wqkv")
            nc.sync.dma_start(wqkv_t[:], wqkv_p[0])
            wproj_t = wp.tile([P, D], dt.bfloat16, name="wproj0", tag="wproj")
            nc.sync.dma_start(wproj_t[:], wproj_p[0])
            biasbc_t = load_biasbc(0)

            ag_out = emit_ag("l0")

            # ---------------- transformer layers ----------------
            for li in range(L):
                wqkv, wproj = wqkv_t, wproj_t
                bproj8, bfc2bc = biasbc_t
                aths = ag_read(ag_out, f"l{li}")
                qkvT0 = qkv_block(aths[0], wqkv, bqkvt[li], 0)
                yT0 = attention_half(qkvT0, 0)
                qkvT1 = qkv_block(aths[1], wqkv, bqkvt[li], 1)
                rsouts = [proj_rs(yT0, wproj, bproj8, 0, f"l{li}p0")]
                yT1 = attention_half(qkvT1, 1)
                rsouts.append(proj_rs(yT1, wproj, bproj8, 1, f"l{li}p1"))

                # prefetch next-layer + MLP weights (emission order => early DMA)
                w1g, w2g = [], []
                for g in range(2):
                    t_ = ws.tile([P, 4, DT, P], dt.bfloat16,
                                 name=f"w1g{li}_{g}", tag="w1g", bufs=2)
                    nc.sync.dma_start(
                        t_[:], wfc1_p[li, 4 * g:4 * g + 4].rearrange(
                            "c p d q -> p c (d q)"))
                    w1g.append(t_)
                if li + 1 < L:
                    wqkv_t = wp.tile([P, DT, QKVC], dt.bfloat16,
                                     name=f"wqkv{li+1}", tag="wqkv")
                    nc.sync.dma_start(wqkv_t[:], wqkv_p[li + 1])
                    wproj_t = wp.tile([P, D], dt.bfloat16, name=f"wproj{li+1}",
                                      tag="wproj")
                    nc.sync.dma_start(wproj_t[:], wproj_p[li + 1])
                    biasbc_t = load_biasbc(li + 1)

                # RS readback, residual add, LN2, transpose to hn2T
                hn2T = app.tile([P, DT, B * TPC], dt.bfloat16, name=f"hn2T{li}",
                                tag="hn2T", bufs=1)
                for h in range(B):
                    rsb = app.tile([TPC, D], dt.bfloat16, name=f"rsb{li}{h}",
                                   tag="rsb")
                    nc.sync.dma_start(rsb[:], rsouts[h][:])
                    nc.vector.tensor_add(hres[h][:], hres[h][:], rsb[:])
                    rstd, mrstd = ln_scale(hres[h][:], f"l{li}m{h}")
                    hn2 = app.tile([TPC, D], dt.bfloat16, name=f"hn2_{li}{h}",
                                   tag="hn")
                    nc.vector.tensor_scalar(hn2[:], hres[h][:], rstd[:],
                                            mrstd[:], OP.mult, OP.add)
                    for dc in range(DT):
                        pst = psA.tile([P, P], dt.bfloat16, space="PSUM",
                                       name=f"pstm_{li}{h}", tag="psA")
                        nc.tensor.transpose(pst[:], hn2[:, dc * P:(dc + 1) * P],
                                            ident_bf[:])
                        nc.vector.tensor_copy(
                            hn2T[:, dc, h * TPC:(h + 1) * TPC], pst[:])

                # ---- MLP fc1 ----
                mTm = app.tile([P, NFC, B * TPC], dt.bfloat16, name=f"mTm{li}",
                               tag="mTm", bufs=1)
                for fc in range(NFC):
                    g, j = fc // 4, fc % 4
                    if j == 0 and g + 2 < 8:
                        t_ = ws.tile([P, 4, DT, P], dt.bfloat16,
                                     name=f"w1g{li}_{g+2}", tag="w1g", bufs=2)
                        nc.sync.dma_start(
                            t_[:], wfc1_p[li, 4 * (g + 2):4 * (g + 2) + 4]
                            .rearrange("c p d q -> p c (d q)"))
                        w1g.append(t_)
                    if j == 0 and g < 2:
                        t_ = ws.tile([P, 4, D], dt.bfloat16,
                                     name=f"w2g{li}_{g}", tag="w2g", bufs=2)
                        nc.sync.dma_start(
                            t_[:], wfc2_p[li, 4 * g:4 * g + 4].rearrange(
                                "c p d -> p c d"))
                        w2g.append(t_)
                    ps = psA.tile([P, 512], dt.float32, space="PSUM",
                                  name="ps_f1", tag="psA")
                    for dti in range(DT):
                        nc.tensor.matmul(
                            ps[:, :B * TPC], lhsT=w1g[g][:, j, dti, :],
                            rhs=hn2T[:, dti, :],
                            start=(dti == 0), stop=(dti == DT - 1))
                    nc.scalar.activation(
                        mTm[:, fc, :], ps[:, :B * TPC], AF.Gelu,
                        bias=bfc1t[li][:, fc:fc + 1])

                # ---- MLP fc2 (stationary mT tiles, token-major out) ----
                ps2 = [[(psP if h == 0 else psS).tile(
                            [P, 512], dt.float32, space="PSUM",
                            name=f"ps_f2_{h}_{dc2}",
                            tag=("psP" if h == 0 else "psS"))
                        for dc2 in range(2)] for h in range(B)]
                for kt in range(NFC):
                    g, j = kt // 4, kt % 4
                    if j == 0 and g + 2 < 8:
                        t_ = ws.tile([P, 4, D], dt.bfloat16,
                                     name=f"w2g{li}_{g+2}", tag="w2g", bufs=2)
                        nc.sync.dma_start(
                            t_[:], wfc2_p[li, 4 * (g + 2):4 * (g + 2) + 4]
                            .rearrange("c p d -> p c d"))
                        w2g.append(t_)
                    for h in range(B):
                        for dc2 in range(2):
                            nc.tensor.matmul(
                                ps2[h][dc2][:],
                                lhsT=mTm[:, kt, h * TPC:(h + 1) * TPC],
                                rhs=w2g[g][:, j, dc2 * 512:(dc2 + 1) * 512],
                                start=(kt == 0), stop=(kt == NFC - 1),
                                skip_group_check=True)
                for h in range(B):
                    for dc2 in range(2):
                        nc.vector.tensor_add(
                            hres[h][:, dc2 * 512:(dc2 + 1) * 512],
                            hres[h][:, dc2 * 512:(dc2 + 1) * 512],
                            ps2[h][dc2][:])
                    nc.vector.tensor_add(hres[h][:], hres[h][:], bfc2bc[:])

                ag_out = emit_ag(f"l{li+1}" if li + 1 < L else "fin")

            # ---------------- LM head ----------------
            warm_rd = cp.tile([1, 2], dt.bfloat16, name="warm_rd")
            nc.sync.dma_start(warm_rd[:], warm_out[0:1, :])
            nc.vector.tensor_add(blm_all[0:1, 0:1], blm_all[0:1, 0:1],
                                 warm_rd[0:1, 0:1])
            afTs = ag_read(ag_out, "fin")
            wlmg = []
            for g in range(2):
                t_ = ws.tile([P, 4, DT, MC], dt.bfloat16, name=f"wlmg{g}",
                             tag="w1g", bufs=2)
                nc.sync.dma_start(
                    t_[:], wlm_p[4 * g:4 * g + 4].rearrange(
                        "c p d m -> p c (d m)"))
                wlmg.append(t_)
            for mc in range(NMC):
                g, j = mc // 4, mc % 4
                if j == 0 and g + 2 < 8:
                    t_ = ws.tile([P, 4, DT, MC], dt.bfloat16, name=f"wlmg{g+2}",
                                 tag="w1g", bufs=2)
                    nc.sync.dma_start(
                        t_[:], wlm_p[4 * (g + 2):4 * (g + 2) + 4].rearrange(
                            "c p d m -> p c (d m)"))
                    wlmg.append(t_)
                psl = [psA.tile([MC, 512], dt.float32, space="PSUM",
                                name=f"ps_lmA{q}", tag="psA") for q in range(2)]
                psl += [psS.tile([MC, 512], dt.float32, space="PSUM",
                                 name=f"ps_lmS{q}", tag="psS") for q in range(2)]
                for dti in range(DT):
                    for q in range(4):
                        h, tk = q // 2, q % 2
                        nc.tensor.matmul(
                            psl[q][:], lhsT=wlmg[g][:, j, dti, :],
                            rhs=afTs[h][:, 4 * tk:4 * tk + 4, dti, :],
                            start=(dti == 0), stop=(dti == DT - 1),
                            skip_group_check=True)
                lsb = app.tile([MC, 4, 512], dt.bfloat16, name="lsb", tag="lsb",
                               bufs=2)
                for q in range(4):
                    nc.vector.tensor_scalar_add(
                        lsb[:, q, :], psl[q][:], blm_all[:, mc:mc + 1])
                nc.gpsimd.dma_start(
                    logits_p[mc * MC:(mc + 1) * MC, :], lsb[:])

    nc.compile()
    return nc


def _get_nc():
    no_coll = os.environ.get("KERNEL_NO_COLL", "0") == "1"
    key = ("nc", no_coll)
    if key not in _CACHE:
        _CACHE[key] = _build_nc(no_coll)
    return _CACHE[key]


def build_in_maps(input_ids, wte, wpe, ln1_g, ln1_b, w_qkv, b_qkv, w_proj,
                  b_proj, ln2_g, ln2_b, w_fc1, b_fc1, w_fc2, b_fc2, lnf_g,
                  lnf_b, w_lm):
    import ml_dtypes
    f32 = np.float32
    bf16 = ml_dtypes.bfloat16

    ids = np.asarray(input_ids).astype(np.int32)
    wte = np.ascontiguousarray(np.asarray(wte, dtype=f32))
    wpe = np.ascontiguousarray(np.asarray(wpe, dtype=f32))
    w_qkv = np.asarray(w_qkv, f32)
    b_qkv = np.asarray(b_qkv, f32)
    w_proj = np.asarray(w_proj, f32)
    b_proj = np.asarray(b_proj, f32)
    w_fc1 = np.asarray(w_fc1, f32)
    b_fc1 = np.asarray(b_fc1, f32)
    w_fc2 = np.asarray(w_fc2, f32)
    b_fc2 = np.asarray(b_fc2, f32)
    w_lm = np.asarray(w_lm, f32)
    g1 = np.asarray(ln1_g, f32)
    b1 = np.asarray(ln1_b, f32)
    g2 = np.asarray(ln2_g, f32)
    b2 = np.asarray(ln2_b, f32)
    gf = np.asarray(lnf_g, f32)
    bf = np.asarray(lnf_b, f32)

    # Fold LN gains into consuming weights; LN biases into consuming biases.
    wqkv_fold = w_qkv * g1[:, :, None]                       # [L, D, 3D]
    bqkv_eff = b_qkv + np.einsum("ld,ldc->lc", b1, w_qkv)    # [L, 3D]
    wfc1_fold = w_fc1 * g2[:, :, None]                       # [L, D, 4D]
    bfc1_eff = b_fc1 + np.einsum("ld,ldf->lf", b2, w_fc1)    # [L, 4D]
    wlm_fold = w_lm * gf[:, None]                            # [D, V]
    blm_eff = bf @ w_lm                                      # [V]

    # Shared (core-independent) stages.
    wfc1_st = np.ascontiguousarray(
        wfc1_fold.reshape(L, DT, P, NFC, P).transpose(0, 3, 2, 1, 4).astype(bf16))
    bfc1_st = np.ascontiguousarray(bfc1_eff.reshape(L, NFC, P).transpose(0, 2, 1))
    wfc2_st = np.ascontiguousarray(w_fc2.reshape(L, NFC, P, D).astype(bf16))
    bproj8_bc = np.ascontiguousarray(
        np.broadcast_to((b_proj / NCORES)[:, None, :], (L, P, D)).astype(bf16))
    bfc2_bc = np.ascontiguousarray(
        np.broadcast_to(b_fc2[:, None, :], (L, P, D)).astype(bf16))

    in_maps = []
    for r in range(NCORES):
        t0, t1 = r * TPC, (r + 1) * TPC
        cols = np.r_[P * r:P * r + P, D + P * r:D + P * r + P,
                     2 * D + P * r:2 * D + P * r + P]
        vs, ve = r * VPC, (r + 1) * VPC
        wqkv_st = np.ascontiguousarray(
            wqkv_fold[:, :, cols].reshape(L, DT, P, QKVC)
            .transpose(0, 2, 1, 3).astype(bf16))
        bqkv_st = np.ascontiguousarray(
            bqkv_eff[:, cols].reshape(L, 3, P).transpose(0, 2, 1))
        wproj_st = np.ascontiguousarray(
            w_proj[:, P * r:P * r + P, :].astype(bf16))
        wlm_pad = np.zeros((D, VPAD), f32)
        wlm_pad[:, :VPC] = wlm_fold[:, vs:ve]
        wlm_st = np.ascontiguousarray(
            wlm_pad.reshape(DT, P, NMC, MC)
            .transpose(2, 1, 0, 3).astype(bf16))
        blm_pad = np.zeros(VPAD, f32)
        blm_pad[:VPC] = blm_eff[vs:ve]
        blm_st = np.ascontiguousarray(blm_pad.reshape(NMC, MC).T)
        in_maps.append({
            "ids_st": np.ascontiguousarray(ids[:, t0:t1].T),
            "wte": wte,
            "wpe_sh": np.ascontiguousarray(wpe[t0:t1]),
            "wqkv_sh": wqkv_st,
            "bqkv_sh": bqkv_st,
            "wproj_sh": wproj_st,
            "bproj8_bc": bproj8_bc,
            "wfc1_st": wfc1_st,
            "bfc1_st": bfc1_st,
            "wfc2_st": wfc2_st,
            "bfc2_bc": bfc2_bc,
            "wlm_st": wlm_st,
            "blm_st": blm_st,
        })

    return in_maps


def kernel(**inputs):
    global last_exec_time_ns, last_result
    from concourse.bass_utils import run_bass_kernel_spmd

    in_maps = build_in_maps(**inputs)
    nc = _get_nc()
    trace = os.environ.get("KERNEL_TRACE", "0") == "1"
    res = run_bass_kernel_spmd(nc, in_maps, list(range(NCORES)), trace=trace)
    last_exec_time_ns = res.exec_time_ns
    last_result = res

    parts = [res.results[r]["logits"][:VPC] for r in range(NCORES)]  # [VPC, B*T]
    full = np.concatenate(parts, axis=0)          # [V, B*T]
    out = np.ascontiguousarray(full.T).reshape(B, T, V).astype(np.float32)
    return out


# revision 41
# speedup vs baseline: 1.0277x; 1.0277x over previous
"""Trainium2 Bass kernel for nn_LuminaLM (4-layer GPT-2-like transformer + LM head).

Strategy: 8-way Megatron tensor parallel with sequence-parallel residual,
TOKEN-MAJOR residual layout.
 - Each core owns 2 of 16 heads, 1/8 of the vocab; MLP is token-local.
 - Residual h is token-sharded AND token-major: core r owns tokens
   [128r,128r+128) of each batch, stored as [128(tokens), 1024(d)] fp32.
   LayerNorm is a per-partition free-axis reduction (bn_stats/bn_aggr on DVE)
   -> no PE stats matmuls, no cross-partition broadcast chains.
 - LN gains/biases folded into consuming weights/biases on the HOST.
 - AllGather payload is rank-major [128p, dt*t] so both the bounce write and
   the gathered readback use large contiguous DMA descriptors; the per-rank
   local transpose (token-major -> feature-major) is 8 cheap PE transposes.
 - qkv: weights stationary, dti-outer so each LDWEIGHTS covers 2 N=512 MMs.
 - Attention: S^T = k^T q per key tile, exp on ScalarE, causal masking via
   affine_select on diagonal tiles, AV with ones-augmented v; normalization
   on yT eviction with a PE-broadcast reciprocal (per half, pipelined).
 - proj / fc2 use the STATIONARY-ACTIVATION trick (lhsT = yT / mT tile,
   weights stream as rhs) so outputs land token-major [t, d]: residual adds,
   RS payloads and LN stay in token-major layout with contiguous DMAs.
 - ReduceScatter per half (2MB bf16), input written with 2KB-contiguous rows.
 - LM head: vocab-sharded; dti-outer accumulation in 4 PSUM banks so each
   LDWEIGHTS covers 4 N=512 streams; logits written bf16 (upcast on host).
Matmuls bf16 with fp32 PSUM accumulation; collectives ride bf16.
"""

import os
import numpy as np

B, T, D, V, L = 2, 1024, 1024, 32000, 4
H, HD = 16, 64
NCORES = 8
P = 128
TPC = T // NCORES          # 128 tokens per core per batch
HPC = H // NCORES          # 2 heads per core
QKVC = 3 * P               # 384 qkv cols per core
VPC = V // NCORES          # 4000 vocab per core
MC = 128                   # lm-head M chunk (full PE width)
NMC = 32                   # 32 chunks of 128 = 4096 (zero-padded past 4000)
VPAD = MC * NMC            # 4096
DT = D // P                # 8 d-tiles
NFC = 4 * D // P           # 32 fc1-output chunks
NT = T // 512              # 2 query chunks of 512
EPS = 1e-5
ATT_SCALE = 1.0 / np.sqrt(HD)
HD1 = HD + 1

_CACHE = {}
last_exec_time_ns = None
last_result = None


def _build_nc(no_coll=False):
    import concourse.bass as bass
    import concourse.mybir as mybir
    import concourse.tile as tile
    from concourse import bacc
    from concourse.masks import make_identity
    from concourse.bass import IndirectOffsetOnAxis

    dt = mybir.dt
    AF = mybir.ActivationFunctionType
    OP = mybir.AluOpType

    nc = bacc.Bacc("TRN2", target_bir_lowering=False, debug=False,
                   num_devices=NCORES)

    # ---- external parameters (per-core shards, staged by host) ----
    ids_p = nc.declare_dram_parameter("ids_st", [TPC, B], dt.int32, isOutput=False)
    wte_p = nc.declare_dram_parameter("wte", [V, D], dt.float32, isOutput=False)
    wpe_p = nc.declare_dram_parameter("wpe_sh", [TPC, D], dt.float32, isOutput=False)
    wqkv_p = nc.declare_dram_parameter("wqkv_sh", [L, P, DT, QKVC], dt.bfloat16, isOutput=False)
    bqkv_p = nc.declare_dram_parameter("bqkv_sh", [L, P, 3], dt.float32, isOutput=False)
    wproj_p = nc.declare_dram_parameter("wproj_sh", [L, P, D], dt.bfloat16, isOutput=False)
    bproj8_p = nc.declare_dram_parameter("bproj8_bc", [L, P, D], dt.bfloat16, isOutput=False)
    wfc1_p = nc.declare_dram_parameter("wfc1_st", [L, NFC, P, DT, P], dt.bfloat16, isOutput=False)
    bfc1_p = nc.declare_dram_parameter("bfc1_st", [L, P, NFC], dt.float32, isOutput=False)
    wfc2_p = nc.declare_dram_parameter("wfc2_st", [L, NFC, P, D], dt.bfloat16, isOutput=False)
    bfc2_p = nc.declare_dram_parameter("bfc2_bc", [L, P, D], dt.bfloat16, isOutput=False)
    wlm_p = nc.declare_dram_parameter("wlm_st", [NMC, P, DT, MC], dt.bfloat16, isOutput=False)
    blm_p = nc.declare_dram_parameter("blm_st", [MC, NMC], dt.float32, isOutput=False)
    logits_p = nc.declare_dram_parameter("logits", [VPAD, B * T], dt.bfloat16, isOutput=True)

    RG = [list(range(NCORES))]

    with tile.TileContext(nc) as tc:
        with (
            tc.tile_pool(name="const", bufs=1) as cp,
            tc.tile_pool(name="wp", bufs=2) as wp,
            tc.tile_pool(name="ws", bufs=2) as ws,
            tc.tile_pool(name="ap", bufs=2) as app,
            tc.tile_pool(name="lnp", bufs=4) as lnp,
            tc.tile_pool(name="psA", bufs=4, space="PSUM") as psA,
            tc.tile_pool(name="psS", bufs=2, space="PSUM") as psS,
            tc.tile_pool(name="psP", bufs=2, space="PSUM") as psP,
            tc.tile_pool(name="dram", bufs=2, space="DRAM") as dramp,
        ):
            # ---------------- warmup collective ----------------
            warm_in = dramp.tile([P, 2], dt.bfloat16, name="warm_in", tag="wrm")
            warm_out = dramp.tile([NCORES * P, 2], dt.bfloat16, name="warm_out",
                                  tag="wrmo", addr_space="Shared")
            warm_sb = cp.tile([P, 2], dt.bfloat16, name="warm_sb")
            nc.vector.memset(warm_sb[:], 0.0)
            nc.sync.dma_start(warm_in[:], warm_sb[:])
            if no_coll:
                nc.sync.dma_start(warm_out[0:P, :], warm_in[:])
            else:
                nc.gpsimd.collective_compute(
                    "AllGather", OP.bypass, replica_groups=RG,
                    ins=[warm_in[:].opt()], outs=[warm_out[:].opt()],
                )

            # ---------------- constants ----------------
            ident_bf = cp.tile([P, P], dt.bfloat16)
            make_identity(nc, ident_bf[:])
            ones_row_bf = cp.tile([1, P], dt.bfloat16)
            nc.vector.memset(ones_row_bf[:], 1.0)
            eps_t = cp.tile([P, 1], dt.float32)
            nc.vector.memset(eps_t[:], EPS)

            # per-layer small bias tiles
            bqkvt, bfc1t = [], []
            for li in range(L):
                t_ = cp.tile([P, 3], dt.float32, name=f"bqkv{li}")
                nc.sync.dma_start(t_[:], bqkv_p[li])
                bqkvt.append(t_)
                t_ = cp.tile([P, NFC], dt.float32, name=f"bfc1{li}")
                nc.sync.dma_start(t_[:], bfc1_p[li])
                bfc1t.append(t_)
            # broadcast residual-bias tiles (bf16, streamed per layer)
            def load_biasbc(li):
                t1_ = ws.tile([P, D], dt.bfloat16, name=f"bproj8_{li}",
                              tag="biasbc", bufs=4)
                nc.sync.dma_start(t1_[:], bproj8_p[li])
                t2_ = ws.tile([P, D], dt.bfloat16, name=f"bfc2_{li}",
                              tag="biasbc", bufs=4)
                nc.sync.dma_start(t2_[:], bfc2_p[li])
                return t1_, t2_
            blm_all = cp.tile([MC, NMC], dt.float32, name="blm_all")
            nc.sync.dma_start(blm_all[:], blm_p[:])

            wpe_tok = cp.tile([TPC, D], dt.float32)
            nc.sync.dma_start(wpe_tok[:], wpe_p[:])
            idx_sb = cp.tile([TPC, B], dt.int32)
            nc.sync.dma_start(idx_sb[:], ids_p[:])

            # residual, token-major fp32
            hres = [cp.tile([TPC, D], dt.float32, name=f"hres{h}") for h in range(B)]

            # ---------------- LN helpers (token-major) ----------------
            def ln_scale(h_ap, name):
                """mean/var over the free (d) axis -> (rstd, mrstd) [P,1]."""
                bs = lnp.tile([P, 2, 6], dt.float32, name=f"bs_{name}", tag="bs")
                hv = h_ap.rearrange("p (c f) -> p c f", f=512)
                for c in range(2):
                    nc.vector.bn_stats(bs[:, c, :], hv[:, c, :])
                ma = lnp.tile([P, 2], dt.float32, name=f"ma_{name}", tag="ma")
                nc.vector.bn_aggr(ma[:], bs[:])
                std = lnp.tile([P, 1], dt.float32, name=f"std_{name}", tag="std")
                nc.scalar.activation(std[:], ma[:, 1:2], AF.Sqrt, bias=eps_t[:])
                rstd = lnp.tile([P, 1], dt.float32, name=f"rstd_{name}", tag="rstd")
                nc.vector.reciprocal_approx_fast(rstd[:], std[:])
                mrstd = lnp.tile([P, 1], dt.float32, name=f"mrstd_{name}", tag="mrstd")
                nc.vector.scalar_tensor_tensor(
                    out=mrstd[:], in0=ma[:, 0:1], scalar=-1.0, in1=rstd[:],
                    op0=OP.mult, op1=OP.mult)
                return rstd, mrstd

            def normalize_transpose(h_ap, hnT, name):
                """LN h_ap [t,d] -> bf16, transpose to hnT [p, dt, t]."""
                rstd, mrstd = ln_scale(h_ap, name)
                hn = app.tile([TPC, D], dt.bfloat16, name=f"hn_{name}", tag="hn")
                nc.vector.tensor_scalar(hn[:], h_ap, rstd[:], mrstd[:],
                                        OP.mult, OP.add)
                for dc in range(DT):
                    pst = psA.tile([P, P], dt.bfloat16, space="PSUM",
                                   name=f"pstn_{name}", tag="psA")
                    nc.tensor.transpose(pst[:], hn[:, dc * P:(dc + 1) * P],
                                        ident_bf[:])
                    nc.vector.tensor_copy(hnT[:, dc, :], pst[:])

            # ---------------- AG (both halves combined) ----------------
            def emit_ag(name):
                """LN1+transpose+bounce+ONE AG; returns ag_out [8P, B*DT*TPC]."""
                ag_in = dramp.tile([P, B, DT * TPC], dt.bfloat16,
                                   name=f"agin_{name}", tag="agin")
                for h in range(B):
                    hnT = app.tile([P, DT, TPC], dt.bfloat16,
                                   name=f"hnT_{name}{h}", tag="hnT")
                    normalize_transpose(hres[h][:], hnT, f"{name}h{h}")
                    nc.sync.dma_start(ag_in[:, h, :], hnT[:])
                ag_out = dramp.tile([NCORES * P, B * DT * TPC], dt.bfloat16,
                                    name=f"agout_{name}", tag="agout",
                                    addr_space="Shared")
                if no_coll:
                    nc.sync.dma_start(ag_out[0:P, :], ag_in[:])
                else:
                    nc.gpsimd.collective_compute(
                        "AllGather", OP.bypass, replica_groups=RG,
                        ins=[ag_in[:].opt()], outs=[ag_out[:].opt()],
                    )
                return ag_out

            def ag_read(ag_out, name):
                """Read gathered acts as aT[h] [p, r, dt, t] (2KB descriptors).
                4 DMAs: rank-groups 0-3 / 4-7 per half so qkv's tk=0 work can
                start while the second half of the payload is still landing."""
                ag_v = ag_out[:].rearrange("(r p) (b f) -> p r b f", p=P, b=B)
                aths = [app.tile([P, NCORES, DT, TPC], dt.bfloat16,
                                 name=f"aT_{name}{h}", tag="aT")
                        for h in range(B)]
                for h in range(B):
                    for rg in range(2):
                        nc.sync.dma_start(
                            aths[h][:, 4 * rg:4 * rg + 4, :, :],
                            ag_v[:, 4 * rg:4 * rg + 4, h, :])
                return aths

            # ---------------- qkv (per half; LDWEIGHTS shared over 2 tk) ----
            def qkv_block(aTh, wqkv, bqkv, half):
                qkvT = app.tile([P, 3, T], dt.bfloat16, name=f"qkvT{half}",
                                tag="qkvT")
                for c in (2, 1, 0):          # v first so v_aug can start early
                    pss = [psP.tile([P, 512], dt.float32, space="PSUM",
                                    name=f"ps_qkv{half}_{c}_{tk}", tag="psP")
                           for tk in range(NT)]
                    for dti in range(DT):
                        for tk in range(NT):
                            nc.tensor.matmul(
                                pss[tk][:],
                                lhsT=wqkv[:, dti, c * P:(c + 1) * P],
                                rhs=aTh[:, 4 * tk:4 * tk + 4, dti, :],
                                start=(dti == 0), stop=(dti == DT - 1),
                                skip_group_check=True)
                    for tk in range(NT):
                        nc.vector.tensor_scalar_add(
                            qkvT[:, c, tk * 512:(tk + 1) * 512], pss[tk][:],
                            bqkv[:, c:c + 1])
                return qkvT

            # ---------------- attention (per half) ----------------
            def attention_half(qkvT, half):
                v_aug = app.tile([P, DT, HPC * HD1], dt.bfloat16,
                                 name=f"vaug{half}", tag="vaug")
                for h2 in range(HPC):
                    nc.vector.memset(
                        v_aug[:, :, h2 * HD1 + HD:h2 * HD1 + HD1], 1.0)
                for tt in range(DT):
                    pst = psA.tile([P, P], dt.bfloat16, space="PSUM",
                                   name=f"pst_v{half}", tag="psA")
                    nc.tensor.transpose(
                        pst[:], qkvT[:, 2, tt * P:(tt + 1) * P], ident_bf[:])
                    for h2 in range(HPC):
                        nc.vector.tensor_copy(
                            v_aug[:, tt, h2 * HD1:h2 * HD1 + HD],
                            pst[:, h2 * HD:(h2 + 1) * HD])

                yT = app.tile([P, T], dt.bfloat16, name=f"yT{half}", tag="yT")
                tails = []

                def emit_tail(ps_y, hs, qc):
                    den = app.tile([1, 512], dt.bfloat16, name="den", tag="den")
                    nc.vector.tensor_copy(den[:], ps_y[HD:HD1, :])
                    ps_bc = psA.tile([HD, 512], dt.float32, space="PSUM",
                                     name="ps_bc", tag="psA")
                    nc.tensor.matmul(ps_bc[:], lhsT=ones_row_bf[:, :HD],
                                     rhs=den[:], start=True, stop=True)
                    recb = app.tile([HD, 512], dt.float32, name="recb", tag="recb")
                    nc.vector.reciprocal_approx_fast(recb[:], ps_bc[:])
                    nc.vector.tensor_tensor(
                        out=yT[hs:hs + HD, qc * 512:(qc + 1) * 512],
                        in0=ps_y[:HD, :], in1=recb[:], op=OP.mult)

                def emit_av(PTt, h2, qc, nkt):
                    ps_y = psA.tile([HD1, 512], dt.float32, space="PSUM",
                                    name="ps_y", tag="psA")
                    for kt in range(nkt):
                        qlo = max(0, kt * P - qc * 512)
                        nc.tensor.matmul(
                            ps_y[:, qlo:512],
                            lhsT=v_aug[:, kt, h2 * HD1:(h2 + 1) * HD1],
                            rhs=PTt[:, kt, qlo:512],
                            start=(kt == 0), stop=(kt == nkt - 1))
                    tails.append((ps_y, h2 * HD, qc))
                    if len(tails) >= 2:
                        emit_tail(*tails.pop(0))

                prev = None
                for qc in range(NT):
                    for h2 in range(HPC):
                        hs = h2 * HD
                        nkt = qc * 4 + 4
                        PTt = app.tile([P, 8, 512], dt.bfloat16,
                                       name=f"PT{half}_{qc}_{h2}", tag="PT",
                                       bufs=2)
                        for kt in range(nkt):
                            qlo = max(0, kt * P - qc * 512)
                            ps_st = psS.tile([P, 512], dt.float32,
                                             space="PSUM", name="ps_st",
                                             tag="psS")
                            nc.tensor.matmul(
                                ps_st[:, qlo:512],
                                lhsT=qkvT[hs:hs + HD, 1, kt * P:(kt + 1) * P],
                                rhs=qkvT[hs:hs + HD, 0,
                                         qc * 512 + qlo:(qc + 1) * 512],
                                start=True, stop=True)
                            nc.scalar.activation(
                                PTt[:, kt, qlo:512], ps_st[:, qlo:512],
                                AF.Exp, scale=ATT_SCALE)
                            if kt >= qc * 4:
                                nc.gpsimd.affine_select(
                                    out=PTt[:, kt, qlo:qlo + P],
                                    in_=PTt[:, kt, qlo:qlo + P],
                                    compare_op=OP.is_ge, fill=0.0, base=0,
                                    pattern=[[1, P]], channel_multiplier=-1)
                        if prev is not None:
                            emit_av(*prev)
                        prev = (PTt, h2, qc, nkt)
                emit_av(*prev)
                while tails:
                    emit_tail(*tails.pop(0))
                return yT

            # ---------------- proj + RS (per half) ----------------
            def proj_rs(yT, wproj, bproj8, half, name):
                """proj partial token-major via stationary-yT; write rs_in."""
                rs_in = dramp.tile([T, D], dt.bfloat16, name=f"rsin_{name}",
                                   tag="rsin")
                for tt in range(DT):
                    prc = app.tile([P, D], dt.bfloat16, name="prc", tag="prc",
                                   bufs=4)
                    for dc2 in range(2):
                        ps = psP.tile([P, 512], dt.float32, space="PSUM",
                                      name="ps_pr", tag="psP")
                        nc.tensor.matmul(
                            ps[:], lhsT=yT[:, tt * P:(tt + 1) * P],
                            rhs=wproj[:, dc2 * 512:(dc2 + 1) * 512],
                            start=True, stop=True)
                        nc.vector.tensor_tensor(
                            out=prc[:, dc2 * 512:(dc2 + 1) * 512], in0=ps[:],
                            in1=bproj8[:, dc2 * 512:(dc2 + 1) * 512],
                            op=OP.add)
                    nc.gpsimd.dma_start(rs_in[tt * P:(tt + 1) * P, :], prc[:])
                rs_out = dramp.tile([TPC, D], dt.bfloat16, name=f"rsout_{name}",
                                    tag="rsout")
                if no_coll:
                    nc.sync.dma_start(rs_out[:], rs_in[0:TPC, :])
                else:
                    nc.gpsimd.collective_compute(
                        "ReduceScatter", OP.add, replica_groups=RG,
                        ins=[rs_in[:].opt()], outs=[rs_out[:].opt()],
                    )
                return rs_out

            # ---------------- embedding ----------------
            for h in range(B):
                nc.gpsimd.indirect_dma_start(
                    out=hres[h][:], out_offset=None, in_=wte_p[:],
                    in_offset=IndirectOffsetOnAxis(ap=idx_sb[:, h:h + 1], axis=0),
                )
                nc.vector.tensor_add(hres[h][:], hres[h][:], wpe_tok[:])

            # weights for layer 0
            wqkv_t = wp.tile([P, DT, QKVC], dt.bfloat16, name="wqkv0", tag="wqkv")
            nc.sync.dma_start(wqkv_t[:], wqkv_p[0])
            wproj_t = wp.tile([P, D], dt.bfloat16, name="wproj0", tag="wproj")
            nc.sync.dma_start(wproj_t[:], wproj_p[0])
            biasbc_t = load_biasbc(0)

            ag_out = emit_ag("l0")

            # ---------------- transformer layers ----------------
            for li in range(L):
                wqkv, wproj = wqkv_t, wproj_t
                bproj8, bfc2bc = biasbc_t
                aths = ag_read(ag_out, f"l{li}")
                qkvT0 = qkv_block(aths[0], wqkv, bqkvt[li], 0)
                yT0 = attention_half(qkvT0, 0)
                qkvT1 = qkv_block(aths[1], wqkv, bqkvt[li], 1)
                rsouts = [proj_rs(yT0, wproj, bproj8, 0, f"l{li}p0")]
                yT1 = attention_half(qkvT1, 1)
                rsouts.append(proj_rs(yT1, wproj, bproj8, 1, f"l{li}p1"))

                # prefetch next-layer + MLP weights (emission order => early DMA)
                w1g, w2g = [], []
                for g in range(2):
                    t_ = ws.tile([P, 4, DT, P], dt.bfloat16,
                                 name=f"w1g{li}_{g}", tag="w1g", bufs=2)
                    nc.sync.dma_start(
                        t_[:], wfc1_p[li, 4 * g:4 * g + 4].rearrange(
                            "c p d q -> p c (d q)"))
                    w1g.append(t_)
                if li + 1 < L:
                    wqkv_t = wp.tile([P, DT, QKVC], dt.bfloat16,
                                     name=f"wqkv{li+1}", tag="wqkv")
                    nc.sync.dma_start(wqkv_t[:], wqkv_p[li + 1])
                    wproj_t = wp.tile([P, D], dt.bfloat16, name=f"wproj{li+1}",
                                      tag="wproj")
                    nc.sync.dma_start(wproj_t[:], wproj_p[li + 1])
                    biasbc_t = load_biasbc(li + 1)

                # RS readback, residual add, LN2, transpose to hn2T
                hn2T = app.tile([P, DT, B * TPC], dt.bfloat16, name=f"hn2T{li}",
                                tag="hn2T", bufs=1)
                for h in range(B):
                    rsb = app.tile([TPC, D], dt.bfloat16, name=f"rsb{li}{h}",
                                   tag="rsb")
                    # scalar queue: a sync-queue rsb would block the MLP
                    # weight streams queued behind it until the RS lands
                    nc.scalar.dma_start(rsb[:], rsouts[h][:])
                    nc.vector.tensor_add(hres[h][:], hres[h][:], rsb[:])
                    rstd, mrstd = ln_scale(hres[h][:], f"l{li}m{h}")
                    hn2 = app.tile([TPC, D], dt.bfloat16, name=f"hn2_{li}{h}",
                                   tag="hn")
                    nc.vector.tensor_scalar(hn2[:], hres[h][:], rstd[:],
                                            mrstd[:], OP.mult, OP.add)
                    for dc in range(DT):
                        pst = psA.tile([P, P], dt.bfloat16, space="PSUM",
                                       name=f"pstm_{li}{h}", tag="psA")
                        nc.tensor.transpose(pst[:], hn2[:, dc * P:(dc + 1) * P],
                                            ident_bf[:])
                        nc.vector.tensor_copy(
                            hn2T[:, dc, h * TPC:(h + 1) * TPC], pst[:])

                # ---- MLP fc1 ----
                mTm = app.tile([P, NFC, B * TPC], dt.bfloat16, name=f"mTm{li}",
                               tag="mTm", bufs=1)
                for fc in range(NFC):
                    g, j = fc // 4, fc % 4
                    if j == 0 and g + 2 < 8:
                        t_ = ws.tile([P, 4, DT, P], dt.bfloat16,
                                     name=f"w1g{li}_{g+2}", tag="w1g", bufs=2)
                        nc.sync.dma_start(
                            t_[:], wfc1_p[li, 4 * (g + 2):4 * (g + 2) + 4]
                            .rearrange("c p d q -> p c (d q)"))
                        w1g.append(t_)
                    if j == 0 and g < 2:
                        t_ = ws.tile([P, 4, D], dt.bfloat16,
                                     name=f"w2g{li}_{g}", tag="w2g", bufs=2)
                        nc.sync.dma_start(
                            t_[:], wfc2_p[li, 4 * g:4 * g + 4].rearrange(
                                "c p d -> p c d"))
                        w2g.append(t_)
                    ps = psA.tile([P, 512], dt.float32, space="PSUM",
                                  name="ps_f1", tag="psA")
                    for dti in range(DT):
                        nc.tensor.matmul(
                            ps[:, :B * TPC], lhsT=w1g[g][:, j, dti, :],
                            rhs=hn2T[:, dti, :],
                            start=(dti == 0), stop=(dti == DT - 1))
                    nc.scalar.activation(
                        mTm[:, fc, :], ps[:, :B * TPC], AF.Gelu,
                        bias=bfc1t[li][:, fc:fc + 1])

                # ---- MLP fc2 (stationary mT tiles, token-major out) ----
                ps2 = [[(psP if h == 0 else psS).tile(
                            [P, 512], dt.float32, space="PSUM",
                            name=f"ps_f2_{h}_{dc2}",
                            tag=("psP" if h == 0 else "psS"))
                        for dc2 in range(2)] for h in range(B)]
                for kt in range(NFC):
                    g, j = kt // 4, kt % 4
                    if j == 0 and g + 2 < 8:
                        t_ = ws.tile([P, 4, D], dt.bfloat16,
                                     name=f"w2g{li}_{g+2}", tag="w2g", bufs=2)
                        nc.sync.dma_start(
                            t_[:], wfc2_p[li, 4 * (g + 2):4 * (g + 2) + 4]
                            .rearrange("c p d -> p c d"))
                        w2g.append(t_)
                    for h in range(B):
                        for dc2 in range(2):
                            nc.tensor.matmul(
                                ps2[h][dc2][:],
                                lhsT=mTm[:, kt, h * TPC:(h + 1) * TPC],
                                rhs=w2g[g][:, j, dc2 * 512:(dc2 + 1) * 512],
                                start=(kt == 0), stop=False,
                                skip_group_check=True)
                for h in range(B):
                    for dc2 in range(2):
                        # fold the fc2 bias into the accumulation group
                        # (rank-1 ones x bias-row) to keep it off the
                        # serial DVE chain before the AllGather
                        nc.tensor.matmul(
                            ps2[h][dc2][:], lhsT=ones_row_bf[0:1, :],
                            rhs=bfc2bc[0:1, dc2 * 512:(dc2 + 1) * 512],
                            start=False, stop=True, skip_group_check=True)
                for h in range(B):
                    for dc2 in range(2):
                        nc.vector.tensor_add(
                            hres[h][:, dc2 * 512:(dc2 + 1) * 512],
                            hres[h][:, dc2 * 512:(dc2 + 1) * 512],
                            ps2[h][dc2][:])

                ag_out = emit_ag(f"l{li+1}" if li + 1 < L else "fin")

            # ---------------- LM head ----------------
            warm_rd = cp.tile([1, 2], dt.bfloat16, name="warm_rd")
            nc.sync.dma_start(warm_rd[:], warm_out[0:1, :])
            nc.vector.tensor_add(blm_all[0:1, 0:1], blm_all[0:1, 0:1],
                                 warm_rd[0:1, 0:1])
            afTs = ag_read(ag_out, "fin")
            wlmg = []
            for g in range(2):
                t_ = ws.tile([P, 4, DT, MC], dt.bfloat16, name=f"wlmg{g}",
                             tag="w1g", bufs=2)
                nc.sync.dma_start(
                    t_[:], wlm_p[4 * g:4 * g + 4].rearrange(
                        "c p d m -> p c (d m)"))
                wlmg.append(t_)
            for mc in range(NMC):
                g, j = mc // 4, mc % 4
                if j == 0 and g + 2 < 8:
                    t_ = ws.tile([P, 4, DT, MC], dt.bfloat16, name=f"wlmg{g+2}",
                                 tag="w1g", bufs=2)
                    nc.sync.dma_start(
                        t_[:], wlm_p[4 * (g + 2):4 * (g + 2) + 4].rearrange(
                            "c p d m -> p c (d m)"))
                    wlmg.append(t_)
                psl = [psA.tile([MC, 512], dt.float32, space="PSUM",
                                name=f"ps_lmA{q}", tag="psA") for q in range(2)]
                psl += [psS.tile([MC, 512], dt.float32, space="PSUM",
                                 name=f"ps_lmS{q}", tag="psS") for q in range(2)]
                for dti in range(DT):
                    for q in range(4):
                        h, tk = q // 2, q % 2
                        nc.tensor.matmul(
                            psl[q][:], lhsT=wlmg[g][:, j, dti, :],
                            rhs=afTs[h][:, 4 * tk:4 * tk + 4, dti, :],
                            start=(dti == 0), stop=(dti == DT - 1),
                            skip_group_check=True)
                lsb = app.tile([MC, 4, 512], dt.bfloat16, name="lsb", tag="lsb",
                               bufs=2)
                for q in range(4):
                    nc.vector.tensor_scalar_add(
                        lsb[:, q, :], psl[q][:], blm_all[:, mc:mc + 1])
                nc.gpsimd.dma_start(
                    logits_p[mc * MC:(mc + 1) * MC, :], lsb[:])

    nc.compile()
    return nc


def _get_nc():
    no_coll = os.environ.get("KERNEL_NO_COLL", "0") == "1"
    key = ("nc", no_coll)
    if key not in _CACHE:
        _CACHE[key] = _build_nc(no_coll)
    return _CACHE[key]


def build_in_maps(input_ids, wte, wpe, ln1_g, ln1_b, w_qkv, b_qkv, w_proj,
                  b_proj, ln2_g, ln2_b, w_fc1, b_fc1, w_fc2, b_fc2, lnf_g,
                  lnf_b, w_lm):
    import ml_dtypes
    f32 = np.float32
    bf16 = ml_dtypes.bfloat16

    ids = np.asarray(input_ids).astype(np.int32)
    wte = np.ascontiguousarray(np.asarray(wte, dtype=f32))
    wpe = np.ascontiguousarray(np.asarray(wpe, dtype=f32))
    w_qkv = np.asarray(w_qkv, f32)
    b_qkv = np.asarray(b_qkv, f32)
    w_proj = np.asarray(w_proj, f32)
    b_proj = np.asarray(b_proj, f32)
    w_fc1 = np.asarray(w_fc1, f32)
    b_fc1 = np.asarray(b_fc1, f32)
    w_fc2 = np.asarray(w_fc2, f32)
    b_fc2 = np.asarray(b_fc2, f32)
    w_lm = np.asarray(w_lm, f32)
    g1 = np.asarray(ln1_g, f32)
    b1 = np.asarray(ln1_b, f32)
    g2 = np.asarray(ln2_g, f32)
    b2 = np.asarray(ln2_b, f32)
    gf = np.asarray(lnf_g, f32)
    bf = np.asarray(lnf_b, f32)

    # Fold LN gains into consuming weights; LN biases into consuming biases.
    wqkv_fold = w_qkv * g1[:, :, None]                       # [L, D, 3D]
    bqkv_eff = b_qkv + np.einsum("ld,ldc->lc", b1, w_qkv)    # [L, 3D]
    wfc1_fold = w_fc1 * g2[:, :, None]                       # [L, D, 4D]
    bfc1_eff = b_fc1 + np.einsum("ld,ldf->lf", b2, w_fc1)    # [L, 4D]
    wlm_fold = w_lm * gf[:, None]                            # [D, V]
    blm_eff = bf @ w_lm                                      # [V]

    # Shared (core-independent) stages.
    wfc1_st = np.ascontiguousarray(
        wfc1_fold.reshape(L, DT, P, NFC, P).transpose(0, 3, 2, 1, 4).astype(bf16))
    bfc1_st = np.ascontiguousarray(bfc1_eff.reshape(L, NFC, P).transpose(0, 2, 1))
    wfc2_st = np.ascontiguousarray(w_fc2.reshape(L, NFC, P, D).astype(bf16))
    bproj8_bc = np.ascontiguousarray(
        np.broadcast_to((b_proj / NCORES)[:, None, :], (L, P, D)).astype(bf16))
    bfc2_bc = np.ascontiguousarray(
        np.broadcast_to(b_fc2[:, None, :], (L, P, D)).astype(bf16))

    in_maps = []
    for r in range(NCORES):
        t0, t1 = r * TPC, (r + 1) * TPC
        cols = np.r_[P * r:P * r + P, D + P * r:D + P * r + P,
                     2 * D + P * r:2 * D + P * r + P]
        vs, ve = r * VPC, (r + 1) * VPC
        wqkv_st = np.ascontiguousarray(
            wqkv_fold[:, :, cols].reshape(L, DT, P, QKVC)
            .transpose(0, 2, 1, 3).astype(bf16))
        bqkv_st = np.ascontiguousarray(
            bqkv_eff[:, cols].reshape(L, 3, P).transpose(0, 2, 1))
        wproj_st = np.ascontiguousarray(
            w_proj[:, P * r:P * r + P, :].astype(bf16))
        wlm_pad = np.zeros((D, VPAD), f32)
        wlm_pad[:, :VPC] = wlm_fold[:, vs:ve]
        wlm_st = np.ascontiguousarray(
            wlm_pad.reshape(DT, P, NMC, MC)
            .transpose(2, 1, 0, 3).astype(bf16))
        blm_pad = np.zeros(VPAD, f32)
        blm_pad[:VPC] = blm_eff[vs:ve]
        blm_st = np.ascontiguousarray(blm_pad.reshape(NMC, MC).T)
        in_maps.append({
            "ids_st": np.ascontiguousarray(ids[:, t0:t1].T),
            "wte": wte,
            "wpe_sh": np.ascontiguousarray(wpe[t0:t1]),
            "wqkv_sh": wqkv_st,
            "bqkv_sh": bqkv_st,
            "wproj_sh": wproj_st,
            "bproj8_bc": bproj8_bc,
            "wfc1_st": wfc1_st,
            "bfc1_st": bfc1_st,
            "wfc2_st": wfc2_st,
            "bfc2_bc": bfc2_bc,
            "wlm_st": wlm_st,
            "blm_st": blm_st,
        })

    return in_maps


def kernel(**inputs):
    global last_exec_time_ns, last_result
    from concourse.bass_utils import run_bass_kernel_spmd

    in_maps = build_in_maps(**inputs)
    nc = _get_nc()
    trace = os.environ.get("KERNEL_TRACE", "0") == "1"
    res = run_bass_kernel_spmd(nc, in_maps, list(range(NCORES)), trace=trace)
    last_exec_time_ns = res.exec_time_ns
    last_result = res

    parts = [res.results[r]["logits"][:VPC] for r in range(NCORES)]  # [VPC, B*T]
    full = np.concatenate(parts, axis=0)          # [V, B*T]
    out = np.ascontiguousarray(full.T).reshape(B, T, V).astype(np.float32)
    return out


# revision 44
# speedup vs baseline: 1.0462x; 1.0180x over previous
"""Trainium2 Bass kernel for nn_LuminaLM (4-layer GPT-2-like transformer + LM head).

Strategy: 8-way Megatron tensor parallel with sequence-parallel residual,
TOKEN-MAJOR residual layout.
 - Each core owns 2 of 16 heads, 1/8 of the vocab; MLP is token-local.
 - Residual h is token-sharded AND token-major: core r owns tokens
   [128r,128r+128) of each batch, stored as [128(tokens), 1024(d)] fp32.
   LayerNorm is a per-partition free-axis reduction (bn_stats/bn_aggr on DVE)
   -> no PE stats matmuls, no cross-partition broadcast chains.
 - LN gains/biases folded into consuming weights/biases on the HOST.
 - AllGather payload is rank-major [128p, dt*t] so both the bounce write and
   the gathered readback use large contiguous DMA descriptors; the per-rank
   local transpose (token-major -> feature-major) is 8 cheap PE transposes.
 - qkv: weights stationary, dti-outer so each LDWEIGHTS covers 2 N=512 MMs.
 - Attention: S^T = k^T q per key tile, exp on ScalarE, causal masking via
   affine_select on diagonal tiles, AV with ones-augmented v; normalization
   on yT eviction with a PE-broadcast reciprocal (per half, pipelined).
 - proj / fc2 use the STATIONARY-ACTIVATION trick (lhsT = yT / mT tile,
   weights stream as rhs) so outputs land token-major [t, d]: residual adds,
   RS payloads and LN stay in token-major layout with contiguous DMAs.
 - ReduceScatter per half (2MB bf16), input written with 2KB-contiguous rows.
 - LM head: vocab-sharded; dti-outer accumulation in 4 PSUM banks so each
   LDWEIGHTS covers 4 N=512 streams; logits written bf16 (upcast on host).
Matmuls bf16 with fp32 PSUM accumulation; collectives ride bf16.
"""

import os
import numpy as np

B, T, D, V, L = 2, 1024, 1024, 32000, 4
H, HD = 16, 64
NCORES = 8
P = 128
TPC = T // NCORES          # 128 tokens per core per batch
HPC = H // NCORES          # 2 heads per core
QKVC = 3 * P               # 384 qkv cols per core
VPC = V // NCORES          # 4000 vocab per core
MC = 128                   # lm-head M chunk (full PE width)
NMC = 32                   # 32 chunks of 128 = 4096 (zero-padded past 4000)
VPAD = MC * NMC            # 4096
DT = D // P                # 8 d-tiles
NFC = 4 * D // P           # 32 fc1-output chunks
NT = T // 512              # 2 query chunks of 512
EPS = 1e-5
ATT_SCALE = 1.0 / np.sqrt(HD)
HD1 = HD + 1

_CACHE = {}
last_exec_time_ns = None
last_result = None


def _build_nc(no_coll=False):
    import concourse.bass as bass
    import concourse.mybir as mybir
    import concourse.tile as tile
    from concourse import bacc
    from concourse.masks import make_identity
    from concourse.bass import IndirectOffsetOnAxis

    dt = mybir.dt
    AF = mybir.ActivationFunctionType
    OP = mybir.AluOpType

    nc = bacc.Bacc("TRN2", target_bir_lowering=False, debug=False,
                   num_devices=NCORES)

    # ---- external parameters (per-core shards, staged by host) ----
    ids_p = nc.declare_dram_parameter("ids_st", [TPC, B], dt.int32, isOutput=False)
    wte_p = nc.declare_dram_parameter("wte", [V, D], dt.float32, isOutput=False)
    wpe_p = nc.declare_dram_parameter("wpe_sh", [TPC, D], dt.float32, isOutput=False)
    wqkv_p = nc.declare_dram_parameter("wqkv_sh", [L, P, DT, QKVC], dt.bfloat16, isOutput=False)
    bqkv_p = nc.declare_dram_parameter("bqkv_sh", [L, P, 3], dt.float32, isOutput=False)
    wproj_p = nc.declare_dram_parameter("wproj_sh", [L, P, D], dt.bfloat16, isOutput=False)
    bproj8_p = nc.declare_dram_parameter("bproj8_bc", [L, P, D], dt.bfloat16, isOutput=False)
    wfc1_p = nc.declare_dram_parameter("wfc1_st", [L, NFC, P, DT, P], dt.bfloat16, isOutput=False)
    bfc1_p = nc.declare_dram_parameter("bfc1_st", [L, P, NFC], dt.float32, isOutput=False)
    wfc2_p = nc.declare_dram_parameter("wfc2_st", [L, NFC, P, D], dt.bfloat16, isOutput=False)
    bfc2_p = nc.declare_dram_parameter("bfc2_bc", [L, P, D], dt.bfloat16, isOutput=False)
    wlm_p = nc.declare_dram_parameter("wlm_st", [NMC, P, DT, MC], dt.bfloat16, isOutput=False)
    blm_p = nc.declare_dram_parameter("blm_st", [MC, NMC], dt.float32, isOutput=False)
    logits_p = nc.declare_dram_parameter("logits", [VPAD, B * T], dt.bfloat16, isOutput=True)

    RG = [list(range(NCORES))]

    with tile.TileContext(nc) as tc:
        with (
            tc.tile_pool(name="const", bufs=1) as cp,
            tc.tile_pool(name="wp", bufs=2) as wp,
            tc.tile_pool(name="ws", bufs=2) as ws,
            tc.tile_pool(name="ap", bufs=2) as app,
            tc.tile_pool(name="lnp", bufs=4) as lnp,
            tc.tile_pool(name="psA", bufs=4, space="PSUM") as psA,
            tc.tile_pool(name="psS", bufs=2, space="PSUM") as psS,
            tc.tile_pool(name="psP", bufs=2, space="PSUM") as psP,
            tc.tile_pool(name="dram", bufs=2, space="DRAM") as dramp,
        ):
            # ---------------- warmup collective ----------------
            warm_in = dramp.tile([P, 2], dt.bfloat16, name="warm_in", tag="wrm")
            warm_out = dramp.tile([NCORES * P, 2], dt.bfloat16, name="warm_out",
                                  tag="wrmo", addr_space="Shared")
            warm_sb = cp.tile([P, 2], dt.bfloat16, name="warm_sb")
            nc.vector.memset(warm_sb[:], 0.0)
            nc.sync.dma_start(warm_in[:], warm_sb[:])
            if no_coll:
                nc.sync.dma_start(warm_out[0:P, :], warm_in[:])
            else:
                nc.gpsimd.collective_compute(
                    "AllGather", OP.bypass, replica_groups=RG,
                    ins=[warm_in[:].opt()], outs=[warm_out[:].opt()],
                )

            # ---------------- constants ----------------
            # idx/wpe first: the embedding gather gates the whole prologue,
            # and every dma_start queued before these delays it
            idx_sb = cp.tile([TPC, B], dt.int32)
            nc.sync.dma_start(idx_sb[:], ids_p[:])
            wpe_tok = cp.tile([TPC, D], dt.float32)
            nc.sync.dma_start(wpe_tok[:], wpe_p[:])

            ident_bf = cp.tile([P, P], dt.bfloat16)
            make_identity(nc, ident_bf[:])
            ones_row_bf = cp.tile([1, P], dt.bfloat16)
            nc.vector.memset(ones_row_bf[:], 1.0)
            eps_t = cp.tile([P, 1], dt.float32)
            nc.vector.memset(eps_t[:], EPS)

            # per-layer small bias tiles
            bqkvt, bfc1t = [], []
            for li in range(L):
                t_ = cp.tile([P, 3], dt.float32, name=f"bqkv{li}")
                nc.sync.dma_start(t_[:], bqkv_p[li])
                bqkvt.append(t_)
                t_ = cp.tile([P, NFC], dt.float32, name=f"bfc1{li}")
                nc.sync.dma_start(t_[:], bfc1_p[li])
                bfc1t.append(t_)
            # broadcast residual-bias tiles (bf16, streamed per layer)
            def load_biasbc(li):
                t1_ = ws.tile([P, D], dt.bfloat16, name=f"bproj8_{li}",
                              tag="biasbc", bufs=4)
                nc.sync.dma_start(t1_[:], bproj8_p[li])
                t2_ = ws.tile([P, D], dt.bfloat16, name=f"bfc2_{li}",
                              tag="biasbc", bufs=4)
                nc.sync.dma_start(t2_[:], bfc2_p[li])
                return t1_, t2_
            blm_all = cp.tile([MC, NMC], dt.float32, name="blm_all")
            nc.sync.dma_start(blm_all[:], blm_p[:])

            # residual, token-major fp32
            hres = [cp.tile([TPC, D], dt.float32, name=f"hres{h}") for h in range(B)]

            # ---------------- LN helpers (token-major) ----------------
            def ln_scale(h_ap, name):
                """mean/var over the free (d) axis -> (rstd, mrstd) [P,1]."""
                bs = lnp.tile([P, 2, 6], dt.float32, name=f"bs_{name}", tag="bs")
                hv = h_ap.rearrange("p (c f) -> p c f", f=512)
                for c in range(2):
                    nc.vector.bn_stats(bs[:, c, :], hv[:, c, :])
                ma = lnp.tile([P, 2], dt.float32, name=f"ma_{name}", tag="ma")
                nc.vector.bn_aggr(ma[:], bs[:])
                std = lnp.tile([P, 1], dt.float32, name=f"std_{name}", tag="std")
                nc.scalar.activation(std[:], ma[:, 1:2], AF.Sqrt, bias=eps_t[:])
                rstd = lnp.tile([P, 1], dt.float32, name=f"rstd_{name}", tag="rstd")
                nc.vector.reciprocal_approx_fast(rstd[:], std[:])
                mrstd = lnp.tile([P, 1], dt.float32, name=f"mrstd_{name}", tag="mrstd")
                nc.vector.scalar_tensor_tensor(
                    out=mrstd[:], in0=ma[:, 0:1], scalar=-1.0, in1=rstd[:],
                    op0=OP.mult, op1=OP.mult)
                return rstd, mrstd

            def normalize_transpose(h_ap, hnT, name):
                """LN h_ap [t,d] -> bf16, transpose to hnT [p, dt, t]."""
                rstd, mrstd = ln_scale(h_ap, name)
                hn = app.tile([TPC, D], dt.bfloat16, name=f"hn_{name}", tag="hn")
                nc.vector.tensor_scalar(hn[:], h_ap, rstd[:], mrstd[:],
                                        OP.mult, OP.add)
                for dc in range(DT):
                    pst = psA.tile([P, P], dt.bfloat16, space="PSUM",
                                   name=f"pstn_{name}", tag="psA")
                    nc.tensor.transpose(pst[:], hn[:, dc * P:(dc + 1) * P],
                                        ident_bf[:])
                    nc.vector.tensor_copy(hnT[:, dc, :], pst[:])

            # ---------------- AG (both halves combined) ----------------
            def emit_ag(name):
                """LN1+transpose+bounce+ONE AG; returns ag_out [8P, B*DT*TPC]."""
                ag_in = dramp.tile([P, B, DT * TPC], dt.bfloat16,
                                   name=f"agin_{name}", tag="agin")
                for h in range(B):
                    hnT = app.tile([P, DT, TPC], dt.bfloat16,
                                   name=f"hnT_{name}{h}", tag="hnT")
                    normalize_transpose(hres[h][:], hnT, f"{name}h{h}")
                    nc.sync.dma_start(ag_in[:, h, :], hnT[:])
                ag_out = dramp.tile([NCORES * P, B * DT * TPC], dt.bfloat16,
                                    name=f"agout_{name}", tag="agout",
                                    addr_space="Shared")
                if no_coll:
                    nc.sync.dma_start(ag_out[0:P, :], ag_in[:])
                else:
                    nc.gpsimd.collective_compute(
                        "AllGather", OP.bypass, replica_groups=RG,
                        ins=[ag_in[:].opt()], outs=[ag_out[:].opt()],
                    )
                return ag_out

            def ag_read(ag_out, name):
                """Read gathered acts as aT[h] [p, r, dt, t] (2KB descriptors).
                4 DMAs: rank-groups 0-3 / 4-7 per half so qkv's tk=0 work can
                start while the second half of the payload is still landing."""
                ag_v = ag_out[:].rearrange("(r p) (b f) -> p r b f", p=P, b=B)
                aths = [app.tile([P, NCORES, DT, TPC], dt.bfloat16,
                                 name=f"aT_{name}{h}", tag="aT")
                        for h in range(B)]
                for h in range(B):
                    for rg in range(2):
                        nc.sync.dma_start(
                            aths[h][:, 4 * rg:4 * rg + 4, :, :],
                            ag_v[:, 4 * rg:4 * rg + 4, h, :])
                return aths

            # ---------------- qkv (per half; LDWEIGHTS shared over 2 tk) ----
            def qkv_block(aTh, wqkv, bqkv, half):
                qkvT = app.tile([P, 3, T], dt.bfloat16, name=f"qkvT{half}",
                                tag="qkvT")
                for c in (2, 1, 0):          # v first so v_aug can start early
                    pss = [psP.tile([P, 512], dt.float32, space="PSUM",
                                    name=f"ps_qkv{half}_{c}_{tk}", tag="psP")
                           for tk in range(NT)]
                    for dti in range(DT):
                        for tk in range(NT):
                            nc.tensor.matmul(
                                pss[tk][:],
                                lhsT=wqkv[:, dti, c * P:(c + 1) * P],
                                rhs=aTh[:, 4 * tk:4 * tk + 4, dti, :],
                                start=(dti == 0), stop=(dti == DT - 1),
                                skip_group_check=True)
                    for tk in range(NT):
                        nc.vector.tensor_scalar_add(
                            qkvT[:, c, tk * 512:(tk + 1) * 512], pss[tk][:],
                            bqkv[:, c:c + 1])
                return qkvT

            # ---------------- attention (per half) ----------------
            def attention_half(qkvT, half):
                v_aug = app.tile([P, DT, HPC * HD1], dt.bfloat16,
                                 name=f"vaug{half}", tag="vaug")
                for h2 in range(HPC):
                    nc.vector.memset(
                        v_aug[:, :, h2 * HD1 + HD:h2 * HD1 + HD1], 1.0)
                for tt in range(DT):
                    pst = psA.tile([P, P], dt.bfloat16, space="PSUM",
                                   name=f"pst_v{half}", tag="psA")
                    nc.tensor.transpose(
                        pst[:], qkvT[:, 2, tt * P:(tt + 1) * P], ident_bf[:])
                    for h2 in range(HPC):
                        nc.vector.tensor_copy(
                            v_aug[:, tt, h2 * HD1:h2 * HD1 + HD],
                            pst[:, h2 * HD:(h2 + 1) * HD])

                yT = app.tile([P, T], dt.bfloat16, name=f"yT{half}", tag="yT")
                tails = []

                def emit_tail(ps_y, hs, qc):
                    den = app.tile([1, 512], dt.bfloat16, name="den", tag="den")
                    nc.vector.tensor_copy(den[:], ps_y[HD:HD1, :])
                    ps_bc = psA.tile([HD, 512], dt.float32, space="PSUM",
                                     name="ps_bc", tag="psA")
                    nc.tensor.matmul(ps_bc[:], lhsT=ones_row_bf[:, :HD],
                                     rhs=den[:], start=True, stop=True)
                    recb = app.tile([HD, 512], dt.float32, name="recb", tag="recb")
                    nc.vector.reciprocal_approx_fast(recb[:], ps_bc[:])
                    nc.vector.tensor_tensor(
                        out=yT[hs:hs + HD, qc * 512:(qc + 1) * 512],
                        in0=ps_y[:HD, :], in1=recb[:], op=OP.mult)

                def emit_av(PTt, h2, qc, nkt):
                    ps_y = psA.tile([HD1, 512], dt.float32, space="PSUM",
                                    name="ps_y", tag="psA")
                    for kt in range(nkt):
                        qlo = max(0, kt * P - qc * 512)
                        nc.tensor.matmul(
                            ps_y[:, qlo:512],
                            lhsT=v_aug[:, kt, h2 * HD1:(h2 + 1) * HD1],
                            rhs=PTt[:, kt, qlo:512],
                            start=(kt == 0), stop=(kt == nkt - 1))
                    tails.append((ps_y, h2 * HD, qc))
                    if len(tails) >= 2:
                        emit_tail(*tails.pop(0))

                prev = None
                for qc in range(NT):
                    for h2 in range(HPC):
                        hs = h2 * HD
                        nkt = qc * 4 + 4
                        PTt = app.tile([P, 8, 512], dt.bfloat16,
                                       name=f"PT{half}_{qc}_{h2}", tag="PT",
                                       bufs=2)
                        for kt in range(nkt):
                            qlo = max(0, kt * P - qc * 512)
                            ps_st = psS.tile([P, 512], dt.float32,
                                             space="PSUM", name="ps_st",
                                             tag="psS")
                            nc.tensor.matmul(
                                ps_st[:, qlo:512],
                                lhsT=qkvT[hs:hs + HD, 1, kt * P:(kt + 1) * P],
                                rhs=qkvT[hs:hs + HD, 0,
                                         qc * 512 + qlo:(qc + 1) * 512],
                                start=True, stop=True)
                            nc.scalar.activation(
                                PTt[:, kt, qlo:512], ps_st[:, qlo:512],
                                AF.Exp, scale=ATT_SCALE)
                            if kt >= qc * 4:
                                nc.gpsimd.affine_select(
                                    out=PTt[:, kt, qlo:qlo + P],
                                    in_=PTt[:, kt, qlo:qlo + P],
                                    compare_op=OP.is_ge, fill=0.0, base=0,
                                    pattern=[[1, P]], channel_multiplier=-1)
                        if prev is not None:
                            emit_av(*prev)
                        prev = (PTt, h2, qc, nkt)
                emit_av(*prev)
                while tails:
                    emit_tail(*tails.pop(0))
                return yT

            # ---------------- proj + RS (per half) ----------------
            def proj_rs(yT, wproj, bproj8, half, name):
                """proj partial token-major via stationary-yT; write rs_in."""
                rs_in = dramp.tile([T, D], dt.bfloat16, name=f"rsin_{name}",
                                   tag="rsin")
                for tt in range(DT):
                    prc = app.tile([P, D], dt.bfloat16, name="prc", tag="prc",
                                   bufs=4)
                    for dc2 in range(2):
                        ps = psP.tile([P, 512], dt.float32, space="PSUM",
                                      name="ps_pr", tag="psP")
                        nc.tensor.matmul(
                            ps[:], lhsT=yT[:, tt * P:(tt + 1) * P],
                            rhs=wproj[:, dc2 * 512:(dc2 + 1) * 512],
                            start=True, stop=True)
                        nc.vector.tensor_tensor(
                            out=prc[:, dc2 * 512:(dc2 + 1) * 512], in0=ps[:],
                            in1=bproj8[:, dc2 * 512:(dc2 + 1) * 512],
                            op=OP.add)
                    nc.gpsimd.dma_start(rs_in[tt * P:(tt + 1) * P, :], prc[:])
                rs_out = dramp.tile([TPC, D], dt.bfloat16, name=f"rsout_{name}",
                                    tag="rsout")
                if no_coll:
                    nc.sync.dma_start(rs_out[:], rs_in[0:TPC, :])
                else:
                    nc.gpsimd.collective_compute(
                        "ReduceScatter", OP.add, replica_groups=RG,
                        ins=[rs_in[:].opt()], outs=[rs_out[:].opt()],
                    )
                return rs_out

            # ---------------- embedding ----------------
            for h in range(B):
                nc.gpsimd.indirect_dma_start(
                    out=hres[h][:], out_offset=None, in_=wte_p[:],
                    in_offset=IndirectOffsetOnAxis(ap=idx_sb[:, h:h + 1], axis=0),
                )
                nc.vector.tensor_add(hres[h][:], hres[h][:], wpe_tok[:])

            # weights for layer 0
            wqkv_t = wp.tile([P, DT, QKVC], dt.bfloat16, name="wqkv0", tag="wqkv")
            nc.sync.dma_start(wqkv_t[:], wqkv_p[0])
            wproj_t = wp.tile([P, D], dt.bfloat16, name="wproj0", tag="wproj")
            nc.sync.dma_start(wproj_t[:], wproj_p[0])
            biasbc_t = load_biasbc(0)

            ag_out = emit_ag("l0")

            # ---------------- transformer layers ----------------
            for li in range(L):
                wqkv, wproj = wqkv_t, wproj_t
                bproj8, bfc2bc = biasbc_t
                aths = ag_read(ag_out, f"l{li}")
                qkvT0 = qkv_block(aths[0], wqkv, bqkvt[li], 0)
                yT0 = attention_half(qkvT0, 0)
                qkvT1 = qkv_block(aths[1], wqkv, bqkvt[li], 1)
                rsouts = [proj_rs(yT0, wproj, bproj8, 0, f"l{li}p0")]
                yT1 = attention_half(qkvT1, 1)
                rsouts.append(proj_rs(yT1, wproj, bproj8, 1, f"l{li}p1"))

                # prefetch next-layer + MLP weights (emission order => early DMA)
                w1g, w2g = [], []
                for g in range(2):
                    t_ = ws.tile([P, 4, DT, P], dt.bfloat16,
                                 name=f"w1g{li}_{g}", tag="w1g", bufs=2)
                    nc.sync.dma_start(
                        t_[:], wfc1_p[li, 4 * g:4 * g + 4].rearrange(
                            "c p d q -> p c (d q)"))
                    w1g.append(t_)
                if li + 1 < L:
                    wqkv_t = wp.tile([P, DT, QKVC], dt.bfloat16,
                                     name=f"wqkv{li+1}", tag="wqkv")
                    nc.sync.dma_start(wqkv_t[:], wqkv_p[li + 1])
                    wproj_t = wp.tile([P, D], dt.bfloat16, name=f"wproj{li+1}",
                                      tag="wproj")
                    nc.sync.dma_start(wproj_t[:], wproj_p[li + 1])
                    biasbc_t = load_biasbc(li + 1)

                # RS readback, residual add, LN2, transpose to hn2T
                hn2T = app.tile([P, DT, B * TPC], dt.bfloat16, name=f"hn2T{li}",
                                tag="hn2T", bufs=1)
                for h in range(B):
                    rsb = app.tile([TPC, D], dt.bfloat16, name=f"rsb{li}{h}",
                                   tag="rsb")
                    # scalar queue: a sync-queue rsb would block the MLP
                    # weight streams queued behind it until the RS lands
                    nc.scalar.dma_start(rsb[:], rsouts[h][:])
                    nc.vector.tensor_add(hres[h][:], hres[h][:], rsb[:])
                    rstd, mrstd = ln_scale(hres[h][:], f"l{li}m{h}")
                    hn2 = app.tile([TPC, D], dt.bfloat16, name=f"hn2_{li}{h}",
                                   tag="hn")
                    nc.vector.tensor_scalar(hn2[:], hres[h][:], rstd[:],
                                            mrstd[:], OP.mult, OP.add)
                    for dc in range(DT):
                        pst = psA.tile([P, P], dt.bfloat16, space="PSUM",
                                       name=f"pstm_{li}{h}", tag="psA")
                        nc.tensor.transpose(pst[:], hn2[:, dc * P:(dc + 1) * P],
                                            ident_bf[:])
                        nc.vector.tensor_copy(
                            hn2T[:, dc, h * TPC:(h + 1) * TPC], pst[:])

                # ---- MLP fc1 ----
                mTm = app.tile([P, NFC, B * TPC], dt.bfloat16, name=f"mTm{li}",
                               tag="mTm", bufs=1)
                for fc in range(NFC):
                    g, j = fc // 4, fc % 4
                    if j == 0 and g + 2 < 8:
                        t_ = ws.tile([P, 4, DT, P], dt.bfloat16,
                                     name=f"w1g{li}_{g+2}", tag="w1g", bufs=2)
                        nc.sync.dma_start(
                            t_[:], wfc1_p[li, 4 * (g + 2):4 * (g + 2) + 4]
                            .rearrange("c p d q -> p c (d q)"))
                        w1g.append(t_)
                    if j == 0 and g < 2:
                        t_ = ws.tile([P, 4, D], dt.bfloat16,
                                     name=f"w2g{li}_{g}", tag="w2g", bufs=2)
                        nc.sync.dma_start(
                            t_[:], wfc2_p[li, 4 * g:4 * g + 4].rearrange(
                                "c p d -> p c d"))
                        w2g.append(t_)
                    ps = psA.tile([P, 512], dt.float32, space="PSUM",
                                  name="ps_f1", tag="psA")
                    for dti in range(DT):
                        nc.tensor.matmul(
                            ps[:, :B * TPC], lhsT=w1g[g][:, j, dti, :],
                            rhs=hn2T[:, dti, :],
                            start=(dti == 0), stop=(dti == DT - 1))
                    nc.scalar.activation(
                        mTm[:, fc, :], ps[:, :B * TPC], AF.Gelu,
                        bias=bfc1t[li][:, fc:fc + 1])

                # ---- MLP fc2 (stationary mT tiles, token-major out) ----
                ps2 = [[(psP if h == 0 else psS).tile(
                            [P, 512], dt.float32, space="PSUM",
                            name=f"ps_f2_{h}_{dc2}",
                            tag=("psP" if h == 0 else "psS"))
                        for dc2 in range(2)] for h in range(B)]
                for kt in range(NFC):
                    g, j = kt // 4, kt % 4
                    if j == 0 and g + 2 < 8:
                        t_ = ws.tile([P, 4, D], dt.bfloat16,
                                     name=f"w2g{li}_{g+2}", tag="w2g", bufs=2)
                        nc.sync.dma_start(
                            t_[:], wfc2_p[li, 4 * (g + 2):4 * (g + 2) + 4]
                            .rearrange("c p d -> p c d"))
                        w2g.append(t_)
                    for h in range(B):
                        for dc2 in range(2):
                            nc.tensor.matmul(
                                ps2[h][dc2][:],
                                lhsT=mTm[:, kt, h * TPC:(h + 1) * TPC],
                                rhs=w2g[g][:, j, dc2 * 512:(dc2 + 1) * 512],
                                start=(kt == 0), stop=False,
                                skip_group_check=True)
                for h in range(B):
                    for dc2 in range(2):
                        # fold the fc2 bias into the accumulation group
                        # (rank-1 ones x bias-row) to keep it off the
                        # serial DVE chain before the AllGather
                        nc.tensor.matmul(
                            ps2[h][dc2][:], lhsT=ones_row_bf[0:1, :],
                            rhs=bfc2bc[0:1, dc2 * 512:(dc2 + 1) * 512],
                            start=False, stop=True, skip_group_check=True)
                for h in range(B):
                    for dc2 in range(2):
                        nc.vector.tensor_add(
                            hres[h][:, dc2 * 512:(dc2 + 1) * 512],
                            hres[h][:, dc2 * 512:(dc2 + 1) * 512],
                            ps2[h][dc2][:])

                ag_out = emit_ag(f"l{li+1}" if li + 1 < L else "fin")

            # ---------------- LM head ----------------
            warm_rd = cp.tile([1, 2], dt.bfloat16, name="warm_rd")
            nc.sync.dma_start(warm_rd[:], warm_out[0:1, :])
            nc.vector.tensor_add(blm_all[0:1, 0:1], blm_all[0:1, 0:1],
                                 warm_rd[0:1, 0:1])
            afTs = ag_read(ag_out, "fin")
            wlmg = []
            for g in range(2):
                t_ = ws.tile([P, 4, DT, MC], dt.bfloat16, name=f"wlmg{g}",
                             tag="w1g", bufs=2)
                nc.sync.dma_start(
                    t_[:], wlm_p[4 * g:4 * g + 4].rearrange(
                        "c p d m -> p c (d m)"))
                wlmg.append(t_)
            for mc in range(NMC):
                g, j = mc // 4, mc % 4
                if j == 0 and g + 2 < 8:
                    t_ = ws.tile([P, 4, DT, MC], dt.bfloat16, name=f"wlmg{g+2}",
                                 tag="w1g", bufs=2)
                    nc.sync.dma_start(
                        t_[:], wlm_p[4 * (g + 2):4 * (g + 2) + 4].rearrange(
                            "c p d m -> p c (d m)"))
                    wlmg.append(t_)
                psl = [psA.tile([MC, 512], dt.float32, space="PSUM",
                                name=f"ps_lmA{q}", tag="psA") for q in range(2)]
                psl += [psS.tile([MC, 512], dt.float32, space="PSUM",
                                 name=f"ps_lmS{q}", tag="psS") for q in range(2)]
                for dti in range(DT):
                    for q in range(4):
                        h, tk = q // 2, q % 2
                        nc.tensor.matmul(
                            psl[q][:], lhsT=wlmg[g][:, j, dti, :],
                            rhs=afTs[h][:, 4 * tk:4 * tk + 4, dti, :],
                            start=(dti == 0), stop=(dti == DT - 1),
                            skip_group_check=True)
                lsb = app.tile([MC, 4, 512], dt.bfloat16, name="lsb", tag="lsb",
                               bufs=2)
                for q in range(4):
                    nc.vector.tensor_scalar_add(
                        lsb[:, q, :], psl[q][:], blm_all[:, mc:mc + 1])
                rows = min(MC, VPC - mc * MC)   # skip zero-padded vocab rows
                nc.gpsimd.dma_start(
                    logits_p[mc * MC:mc * MC + rows, :], lsb[0:rows])

    nc.compile()
    return nc


def _get_nc():
    no_coll = os.environ.get("KERNEL_NO_COLL", "0") == "1"
    key = ("nc", no_coll)
    if key not in _CACHE:
        _CACHE[key] = _build_nc(no_coll)
    return _CACHE[key]


def build_in_maps(input_ids, wte, wpe, ln1_g, ln1_b, w_qkv, b_qkv, w_proj,
                  b_proj, ln2_g, ln2_b, w_fc1, b_fc1, w_fc2, b_fc2, lnf_g,
                  lnf_b, w_lm):
    import ml_dtypes
    f32 = np.float32
    bf16 = ml_dtypes.bfloat16

    ids = np.asarray(input_ids).astype(np.int32)
    wte = np.ascontiguousarray(np.asarray(wte, dtype=f32))
    wpe = np.ascontiguousarray(np.asarray(wpe, dtype=f32))
    w_qkv = np.asarray(w_qkv, f32)
    b_qkv = np.asarray(b_qkv, f32)
    w_proj = np.asarray(w_proj, f32)
    b_proj = np.asarray(b_proj, f32)
    w_fc1 = np.asarray(w_fc1, f32)
    b_fc1 = np.asarray(b_fc1, f32)
    w_fc2 = np.asarray(w_fc2, f32)
    b_fc2 = np.asarray(b_fc2, f32)
    w_lm = np.asarray(w_lm, f32)
    g1 = np.asarray(ln1_g, f32)
    b1 = np.asarray(ln1_b, f32)
    g2 = np.asarray(ln2_g, f32)
    b2 = np.asarray(ln2_b, f32)
    gf = np.asarray(lnf_g, f32)
    bf = np.asarray(lnf_b, f32)

    # Fold LN gains into consuming weights; LN biases into consuming biases.
    wqkv_fold = w_qkv * g1[:, :, None]                       # [L, D, 3D]
    bqkv_eff = b_qkv + np.einsum("ld,ldc->lc", b1, w_qkv)    # [L, 3D]
    wfc1_fold = w_fc1 * g2[:, :, None]                       # [L, D, 4D]
    bfc1_eff = b_fc1 + np.einsum("ld,ldf->lf", b2, w_fc1)    # [L, 4D]
    wlm_fold = w_lm * gf[:, None]                            # [D, V]
    blm_eff = bf @ w_lm                                      # [V]

    # Shared (core-independent) stages.
    wfc1_st = np.ascontiguousarray(
        wfc1_fold.reshape(L, DT, P, NFC, P).transpose(0, 3, 2, 1, 4).astype(bf16))
    bfc1_st = np.ascontiguousarray(bfc1_eff.reshape(L, NFC, P).transpose(0, 2, 1))
    wfc2_st = np.ascontiguousarray(w_fc2.reshape(L, NFC, P, D).astype(bf16))
    bproj8_bc = np.ascontiguousarray(
        np.broadcast_to((b_proj / NCORES)[:, None, :], (L, P, D)).astype(bf16))
    bfc2_bc = np.ascontiguousarray(
        np.broadcast_to(b_fc2[:, None, :], (L, P, D)).astype(bf16))

    in_maps = []
    for r in range(NCORES):
        t0, t1 = r * TPC, (r + 1) * TPC
        cols = np.r_[P * r:P * r + P, D + P * r:D + P * r + P,
                     2 * D + P * r:2 * D + P * r + P]
        vs, ve = r * VPC, (r + 1) * VPC
        wqkv_st = np.ascontiguousarray(
            wqkv_fold[:, :, cols].reshape(L, DT, P, QKVC)
            .transpose(0, 2, 1, 3).astype(bf16))
        bqkv_st = np.ascontiguousarray(
            bqkv_eff[:, cols].reshape(L, 3, P).transpose(0, 2, 1))
        wproj_st = np.ascontiguousarray(
            w_proj[:, P * r:P * r + P, :].astype(bf16))
        wlm_pad = np.zeros((D, VPAD), f32)
        wlm_pad[:, :VPC] = wlm_fold[:, vs:ve]
        wlm_st = np.ascontiguousarray(
            wlm_pad.reshape(DT, P, NMC, MC)
            .transpose(2, 1, 0, 3).astype(bf16))
        blm_pad = np.zeros(VPAD, f32)
        blm_pad[:VPC] = blm_eff[vs:ve]
        blm_st = np.ascontiguousarray(blm_pad.reshape(NMC, MC).T)
        in_maps.append({
            "ids_st": np.ascontiguousarray(ids[:, t0:t1].T),
            "wte": wte,
            "wpe_sh": np.ascontiguousarray(wpe[t0:t1]),
            "wqkv_sh": wqkv_st,
            "bqkv_sh": bqkv_st,
            "wproj_sh": wproj_st,
            "bproj8_bc": bproj8_bc,
            "wfc1_st": wfc1_st,
            "bfc1_st": bfc1_st,
            "wfc2_st": wfc2_st,
            "bfc2_bc": bfc2_bc,
            "wlm_st": wlm_st,
            "blm_st": blm_st,
        })

    return in_maps


def kernel(**inputs):
    global last_exec_time_ns, last_result
    from concourse.bass_utils import run_bass_kernel_spmd

    in_maps = build_in_maps(**inputs)
    nc = _get_nc()
    trace = os.environ.get("KERNEL_TRACE", "0") == "1"
    res = run_bass_kernel_spmd(nc, in_maps, list(range(NCORES)), trace=trace)
    last_exec_time_ns = res.exec_time_ns
    last_result = res

    parts = [res.results[r]["logits"][:VPC] for r in range(NCORES)]  # [VPC, B*T]
    full = np.concatenate(parts, axis=0)          # [V, B*T]
    out = np.ascontiguousarray(full.T).reshape(B, T, V).astype(np.float32)
    return out
